# revision 1
# baseline (speedup 1.0000x reference)
"""GAT (3-layer, PyG-style) Trainium2 Bass kernel, 8-core dst-sharded.

Self-contained: takes full inputs, shards internally, returns full output.

Design:
  - dst nodes sharded across 8 cores (graph parallel per the sharding hint).
  - Per layer (3 SPMD launches; host only shards/permutes/transposes/casts
    between them):
    dense phase: node-major DRAM gather-table rows [hW*bn_s (bf16) | al_s f32]
      built by PE matmuls (lhsT = h^T chunks, rhs = W_aug), plus dense al_d.
    edge phase: padded-CSR slots (K slots per dst per src-half, K bucketed by
      max src-half degree), dma_gather of one 256B/512B table row per slot,
      ex = exp(leaky_relu(al_s + al_d)), messages scaled by ex, segment-sum
      via fixed shifted selection-matrix matmuls accumulating
      [msg_sums | sum_ex] per 128-dst window in PSUM; drain divides by
      sum_ex and adds skip matmul + bias (+BN fold, +ReLU).
  - src space is split in two halves with separate table bases so dma_gather's
    int16 indices stay < 32768; padded slots gather a sentinel row whose
    al_s = -40 (=> ex ~ 0) and whose message contribution ~ 0.
"""
import numpy as np
import ml_dtypes

import concourse.bacc as bacc
import concourse.mybir as mybir
import concourse.tile as tile
from concourse.alu_op_type import AluOpType
from concourse.bass_utils import run_bass_kernel_spmd

BF16 = mybir.dt.bfloat16
F32 = mybir.dt.float32
I16 = mybir.dt.int16

NC = 8
KLIST = (8, 16, 32, 64, 128)
P = 128
EPS = 1e-5
SENT_ALS = -40.0


def _round_up(x, m):
    return (x + m - 1) // m * m


# ----------------------------------------------------------------- planning

def build_plan(src, dst, N):
    D = N // NC
    HALF = N // 2
    core = dst // D
    dloc = dst % D
    half = (src >= HALF).astype(np.int64)

    deg = np.zeros((NC, D, 2), np.int64)
    np.add.at(deg, (core, dloc, half), 1)
    mx = deg.max(axis=2)  # [NC, D]
    Kd = np.select([mx <= 8, mx <= 16, mx <= 32, mx <= 64], [8, 16, 32, 64], 128)
    assert mx.max() <= 128, f"degree bucket overflow: {mx.max()}"

    nK = {k: _round_up(int((Kd == k).sum(axis=1).max()), 16) for k in KLIST}
    Dp = sum(nK.values())
    nK[8] += (-Dp) % 128
    Dp = sum(nK.values())
    off = {}
    o = 0
    for k in KLIST:
        off[k] = o
        o += nK[k]

    slabs = []
    for k in KLIST:
        q = P // k
        for i in range(nK[k] * k // P):
            slabs.append((k, off[k] + i * q))
    nslab = len(slabs)

    slab_win = [vd0 // P for (_, vd0) in slabs]
    first_slab = {}
    last_slab = {}
    for i, w in enumerate(slab_win):
        first_slab.setdefault(w, i)
        last_slab[w] = i
    nwin = Dp // P

    slot0 = np.zeros(Dp, np.int64)
    for si, (k, vd0) in enumerate(slabs):
        q = P // k
        for j in range(q):
            slot0[vd0 + j] = si * P + j * k
    TOT = _round_up(nslab, 64) * P

    shared = dict(N=N, D=D, HALF=HALF, Dp=Dp, nK=nK, off=off, slabs=slabs,
                  slab_win=slab_win, first_slab=first_slab,
                  last_slab=last_slab, nwin=nwin, TOT=TOT)

    plans = []
    for c in range(NC):
        vid = np.full(D, -1, np.int64)
        vmap = np.full(Dp, -1, np.int64)
        used = {k: 0 for k in KLIST}
        order = np.argsort(Kd[c], kind="stable")
        for d in order:
            k = int(Kd[c, d])
            pos = off[k] + used[k]
            used[k] += 1
            vid[d] = pos
            vmap[pos] = d
        em = core == c
        es = src[em]
        evd = vid[dloc[em]]
        eh = half[em]
        key = evd * 2 + eh
        si = np.argsort(key, kind="stable")
        ks = key[si]
        starts = np.zeros(2 * Dp + 1, np.int64)
        np.cumsum(np.bincount(ks, minlength=2 * Dp), out=starts[1:])
        rank = np.arange(len(ks)) - starts[ks]
        spos = slot0[evd[si]] + rank
        essorted = es[si]
        lo = np.full(TOT, HALF, np.int64)
        hi = np.full(TOT, HALF, np.int64)
        mlo = ks % 2 == 0
        lo[spos[mlo]] = essorted[mlo]
        hi[spos[~mlo]] = essorted[~mlo] - HALF
        plans.append(dict(vmap=vmap, idx_lo=_wrap16(lo), idx_hi=_wrap16(hi)))
    return shared, plans


def _wrap16(stream):
    TOT = len(stream)
    w = stream.reshape(TOT // 16, 16).T.astype(np.int16)
    return np.tile(w, (8, 1))


def _svar_layout():
    """index map + host array of shifted selection matrices."""
    idx = {}
    mats = []
    for k in KLIST:
        q = P // k
        for s in range(P // q):
            idx[(k, s)] = len(mats)
            m = np.zeros((P, P), np.float32)
            for p in range(P):
                m[p, s * q + p // k] = 1.0
            mats.append(m)
    return idx, np.concatenate(mats, 0)


SVAR_INDEX, SVAR_NP = _svar_layout()
NSV = len(SVAR_INDEX)


# ------------------------------------------------------------- kernel build

def build_layer(shared, F, OUTW, relu, mean_heads, tcap, gb, dbg_stage=99):
    N, HALF, Dp = shared["N"], shared["HALF"], shared["Dp"]
    nwin = shared["nwin"]
    slabs = shared["slabs"]
    slab_win = shared["slab_win"]
    first_slab, last_slab = shared["first_slab"], shared["last_slab"]
    TOT = shared["TOT"]
    nslab = len(slabs)
    assert 64 % tcap == 0 or tcap % 64 == 0
    groups = []
    s0 = 0
    while s0 < nslab:
        groups.append((s0, min(s0 + tcap, nslab)))
        s0 += tcap

    RW = 128 if OUTW == 64 else 256
    NA = 72 if OUTW == 64 else 260
    NAW = OUTW + 4
    PADW = 128 if OUTW == 64 else 512
    Npad = _round_up(N, P)
    nchunk = Npad // P
    TROWS = Npad + 2

    nc = bacc.Bacc("TRN2", target_bir_lowering=False, debug=False)
    hT = nc.dram_tensor("hT", [F, Npad], F32, kind="ExternalInput")
    hTow = nc.dram_tensor("hTow", [F, Dp], F32, kind="ExternalInput")
    Waug = nc.dram_tensor("Waug", [F, NA], F32, kind="ExternalInput")
    skipW = nc.dram_tensor("skipW", [F, 64], F32, kind="ExternalInput")
    biasR = nc.dram_tensor("biasR", [P, 64], F32, kind="ExternalInput")
    svar_in = nc.dram_tensor("svar", [NSV * P, P], BF16, kind="ExternalInput")
    rep_in = nc.dram_tensor("rep", [len(KLIST) * 16, P], F32, kind="ExternalInput")
    sent_in = nc.dram_tensor("sent", [2, RW], BF16, kind="ExternalInput")
    idx_lo = nc.dram_tensor("idx_lo", [P, TOT // 16], I16, kind="ExternalInput")
    idx_hi = nc.dram_tensor("idx_hi", [P, TOT // 16], I16, kind="ExternalInput")
    a2s_in = (nc.dram_tensor("a2s", [P, 1, 256], BF16, kind="ExternalInput")
              if mean_heads else None)

    table = nc.dram_tensor("table", [TROWS, RW], BF16, kind="Internal")
    aldv_d = nc.dram_tensor("aldv", [Dp, 4], F32, kind="Internal")
    y_out = nc.dram_tensor("y", [Dp, 64], F32, kind="ExternalOutput")

    def table_row_ranges(n0, n1):
        out = []
        cuts = sorted({n0, min(max(HALF, n0), n1), n1})
        for a, b in zip(cuts, cuts[1:]):
            if a >= b:
                continue
            out.append((a, b, a if a < HALF else a + 1))
        return out

    with tile.TileContext(nc) as tc:
        with (
            tc.tile_pool(name="const", bufs=1) as cp,
            tc.tile_pool(name="ybuf", bufs=1) as yp,
        ):
            waug_sb = cp.tile([F, NA], F32)
            nc.sync.dma_start(waug_sb[:], Waug[:])
            skipw_sb = cp.tile([F, 64], F32)
            nc.sync.dma_start(skipw_sb[:], skipW[:])
            bias_sb = cp.tile([P, 64], F32)
            nc.sync.dma_start(bias_sb[:], biasR[:])
            svar_sb = cp.tile([P, NSV, P], BF16)
            nc.sync.dma_start(svar_sb[:],
                              svar_in[:].rearrange("(v p) c -> p v c", p=P))
            rep_sb = cp.tile([16, len(KLIST), P], F32)
            nc.sync.dma_start(rep_sb[:],
                              rep_in[:].rearrange("(v p) c -> p v c", p=16))
            hTow_sb = cp.tile([F, Dp], F32)
            nc.sync.dma_start(hTow_sb[:], hTow[:])
            if mean_heads:
                a2s_sb = cp.tile([P, 1, 256], BF16)
                nc.sync.dma_start(a2s_sb[:], a2s_in[:])
            y_sb = yp.tile([P, nwin, 64], F32)
            nc.gpsimd.memset(y_sb[:], 0)

            # ---------------- dense phase: gather table + dense al_d
            with (
                tc.tile_pool(name="dstage", bufs=3) as dsp,
                tc.tile_pool(name="pdense", bufs=2, space="PSUM") as pd,
                tc.tile_pool(name="pal", bufs=1, space="PSUM") as pal,
            ):
                for g0 in range(0, nchunk, gb):
                    g1 = min(g0 + gb, nchunk)
                    ng = g1 - g0
                    stage = dsp.tile([F, gb * P], F32, tag="stage")
                    nc.sync.dma_start(stage[:, : ng * P], hT[:, g0 * P:g1 * P])
                    dps = pd.tile([P, gb * PADW], F32, space="PSUM", tag="dps")
                    for i in range(ng):
                        nc.tensor.matmul(
                            dps[:, i * PADW: i * PADW + NA],
                            stage[:, i * P: (i + 1) * P],
                            waug_sb[:],
                            start=True, stop=True,
                        )
                    tstage = dsp.tile([P, gb, RW], BF16, tag="tstage")
                    nc.gpsimd.memset(tstage[:], 0)
                    dv = dps[:].rearrange("p (i w) -> p i w", w=PADW)
                    nc.vector.tensor_copy(tstage[:, :ng, 0:OUTW],
                                          dv[:, :ng, 0:OUTW])
                    if not mean_heads:
                        tf32 = tstage[:].bitcast(F32)
                        nc.vector.tensor_copy(tf32[:, :ng, 32:36],
                                              dv[:, :ng, 64:68])
                    for (a, b, r) in table_row_ranges(g0 * P, g1 * P):
                        # emit aligned middle as one DMA; partial chunks solo
                        n0 = a
                        while n0 < b:
                            if n0 % P != 0 or b - n0 < P:
                                n1 = min(b, n0 - n0 % P + P)
                                ci = n0 // P - g0
                                nc.sync.dma_start(
                                    table[r + n0 - a: r + n1 - a, :],
                                    tstage[n0 % P: n0 % P + (n1 - n0), ci, :],
                                )
                            else:
                                n1 = n0 + (b - n0) // P * P
                                ci = n0 // P - g0
                                m = (n1 - n0) // P
                                nc.sync.dma_start(
                                    table[r + n0 - a: r + n1 - a, :].rearrange(
                                        "(i p) w -> p i w", p=P),
                                    tstage[:, ci: ci + m, :],
                                )
                            n0 = n1
                sent_sb = dsp.tile([2, RW], BF16, tag="sent")
                nc.sync.dma_start(sent_sb[:], sent_in[:])
                nc.sync.dma_start(table[HALF: HALF + 1, :], sent_sb[0:1, :])
                nc.sync.dma_start(table[N + 1: N + 2, :], sent_sb[1:2, :])

                ndc = Dp // P
                alps = pal.tile([P, ndc * 4], F32, space="PSUM")
                for i in range(ndc):
                    nc.tensor.matmul(
                        alps[:, i * 4: (i + 1) * 4],
                        hTow_sb[:, i * P: (i + 1) * P],
                        waug_sb[:, NA - 4: NA],
                        start=True, stop=True,
                    )
                alsb = dsp.tile([P, ndc * 4], F32, tag="alsb")
                nc.vector.tensor_copy(alsb[:], alps[:])
                nc.sync.dma_start(
                    aldv_d[:].rearrange("(i p) h -> p i h", p=P),
                    alsb[:].rearrange("p (i h) -> p i h", h=4),
                )

            # ---------------- edge phase
            with (
                tc.tile_pool(name="gpool", bufs=2) as gp,
                tc.tile_pool(name="spool", bufs=2) as ssp,
                tc.tile_pool(name="pwin", bufs=3, space="PSUM") as pw,
                tc.tile_pool(name="pex", bufs=3 if mean_heads else 1,
                             space="PSUM") as px,
                tc.tile_pool(name="palde", bufs=1 if mean_heads else 2,
                             space="PSUM") as pa,
                tc.tile_pool(name="psk", bufs=1 if mean_heads else 2,
                             space="PSUM") as pk,
            ):
                win_ps = {}
                ex_ps = {}
                for (s0, s1) in groups:
                    T = s1 - s0
                    g_lo = gp.tile([P, tcap, RW], BF16, tag="Glo")
                    g_hi = gp.tile([P, tcap, RW], BF16, tag="Ghi")
                    il_t = ssp.tile([P, tcap * 8], I16, tag="il")
                    ih_t = ssp.tile([P, tcap * 8], I16, tag="ih")
                    nc.sync.dma_start(il_t[:], idx_lo[:, s0 * 8:(s0 + tcap) * 8])
                    nc.sync.dma_start(ih_t[:], idx_hi[:, s0 * 8:(s0 + tcap) * 8])
                    if dbg_stage < 2:
                        continue
                    nc.gpsimd.dma_gather(
                        g_lo[:], table[0: HALF + 1, :],
                        il_t[:], tcap * P, tcap * P, RW,
                        single_packet=False)
                    nc.gpsimd.dma_gather(
                        g_hi[:], table[HALF + 1: TROWS, :],
                        ih_t[:], tcap * P, tcap * P, RW,
                        single_packet=False)

                    if dbg_stage < 3:
                        continue
                    alde = ssp.tile([P, tcap, 4], F32, tag="alde")
                    i = s0
                    while i < s1:
                        k = slabs[i][0]
                        j = i
                        while j < s1 and slabs[j][0] == k:
                            j += 1
                        q = P // k
                        run = j - i
                        vb = slabs[i][1]
                        cont = ssp.tile([16, tcap, 4], F32, tag="cont")
                        nc.sync.dma_start(
                            cont[:q, :run, :],
                            aldv_d[vb: vb + run * q, :].rearrange(
                                "(t j) h -> j t h", j=q),
                        )
                        aps = pa.tile([P, tcap * 4], F32, space="PSUM",
                                      tag="aldeps")
                        nc.tensor.matmul(
                            aps[:, : run * 4],
                            rep_sb[:q, KLIST.index(k), :],
                            cont[:q, :run, :].rearrange("j t h -> j (t h)"),
                            start=True, stop=True,
                        )
                        nc.vector.tensor_copy(
                            alde[:, i - s0: j - s0, :],
                            aps[:, : run * 4].rearrange("p (t h) -> p t h", h=4),
                        )
                        i = j

                    if dbg_stage < 4:
                        continue
                    if mean_heads:
                        ex_t = ssp.tile([P, 2 * tcap, 4], BF16, tag="ex")
                        tmp_t = ssp.tile([P, tcap // 2, 256], BF16, tag="tmp")
                    z_t = ssp.tile([P, 2 * tcap, 4], F32, tag="z")
                    for h in range(2):
                        gs = (g_lo if h == 0 else g_hi)[:, :T, :]
                        zs = z_t[:, h * tcap: h * tcap + T, :]
                        if mean_heads:
                            hc = tcap // 2
                            for c0 in range(0, T, hc):
                                c1 = min(T, c0 + hc)
                                nc.vector.tensor_tensor(
                                    tmp_t[:, : c1 - c0, :], gs[:, c0:c1, :],
                                    a2s_sb[:].to_broadcast([P, c1 - c0, 256]),
                                    AluOpType.mult)
                                nc.vector.reduce_sum(
                                    zs[:, c0:c1, :],
                                    tmp_t[:, : c1 - c0, :].rearrange(
                                        "p t (g c) -> p t g c", c=64),
                                    axis=mybir.AxisListType.X)
                        else:
                            gf = (g_lo if h == 0 else g_hi)[:].bitcast(F32)
                            nc.vector.tensor_copy(zs, gf[:, :T, 32:36])
                        nc.vector.tensor_tensor(zs, zs, alde[:, :T, :],
                                                AluOpType.add)
                        nc.vector.scalar_tensor_tensor(
                            zs, zs, 0.2, zs, AluOpType.mult, AluOpType.max)
                        if mean_heads:
                            nc.scalar.activation(
                                ex_t[:, h * tcap: h * tcap + T, :], zs,
                                mybir.ActivationFunctionType.Exp)
                        else:
                            nc.scalar.activation(
                                gs[:, :, 64:68], zs,
                                mybir.ActivationFunctionType.Exp)
                        for hh in range(4):
                            ex_ap = (ex_t[:, h * tcap: h * tcap + T, hh:hh + 1]
                                     if mean_heads
                                     else gs[:, :, 64 + hh: 65 + hh])
                            nc.vector.tensor_tensor(
                                gs[:, :, hh * (OUTW // 4):
                                   (hh + 1) * (OUTW // 4)],
                                gs[:, :, hh * (OUTW // 4):
                                   (hh + 1) * (OUTW // 4)],
                                ex_ap.to_broadcast([P, T, OUTW // 4]),
                                AluOpType.mult,
                            )

                    if dbg_stage < 5:
                        continue
                    for i in range(s0, s1):
                        k, vd0 = slabs[i]
                        w = slab_win[i]
                        if w not in win_ps:
                            win_ps[w] = pw.tile([P, NAW], F32, space="PSUM",
                                                tag="win", name=f"win{w}")
                            if mean_heads:
                                ex_ps[w] = px.tile([P, 4], F32, space="PSUM",
                                                   tag="exw", name=f"exw{w}")
                        q = P // k
                        sv = svar_sb[:, SVAR_INDEX[(k, (vd0 % P) // q)], :]
                        for h in range(2):
                            st = (h == 0) and (first_slab[w] == i)
                            fin = (h == 1) and (last_slab[w] == i)
                            gh = g_lo if h == 0 else g_hi
                            t = i - s0
                            if mean_heads:
                                nc.tensor.matmul(
                                    win_ps[w][:, 0:OUTW], sv, gh[:, t, :],
                                    start=st, stop=fin, skip_group_check=True)
                                nc.tensor.matmul(
                                    ex_ps[w][:], sv,
                                    ex_t[:, h * tcap + t, :],
                                    start=st, stop=fin, skip_group_check=True)
                            else:
                                nc.tensor.matmul(
                                    win_ps[w][:, 0:NAW], sv, gh[:, t, 0:NAW],
                                    start=st, stop=fin, skip_group_check=True)
                        if last_slab[w] != i or dbg_stage < 6:
                            continue
                        pwin = win_ps.pop(w)
                        pex = ex_ps.pop(w) if mean_heads else None
                        sk = pk.tile([P, 64], F32, space="PSUM", tag="skps")
                        nc.tensor.matmul(
                            sk[:], hTow_sb[:, w * P: (w + 1) * P], skipw_sb[:],
                            start=True, stop=True)
                        rec = ssp.tile([P, 4], F32, tag="rec")
                        nc.vector.reciprocal(
                            rec[:], pex[:] if mean_heads else pwin[:, OUTW:NAW])
                        yw = y_sb[:, w, :]
                        if mean_heads:
                            m_t = ssp.tile([P, 4, 64], F32, tag="mt")
                            for hh in range(4):
                                nc.vector.tensor_tensor(
                                    m_t[:, hh, :],
                                    pwin[:, hh * 64: (hh + 1) * 64],
                                    rec[:, hh: hh + 1].to_broadcast([P, 64]),
                                    AluOpType.mult)
                            nc.vector.tensor_tensor(yw, m_t[:, 0, :],
                                                    m_t[:, 1, :], AluOpType.add)
                            nc.vector.tensor_tensor(yw, yw, m_t[:, 2, :],
                                                    AluOpType.add)
                            nc.vector.tensor_tensor(yw, yw, m_t[:, 3, :],
                                                    AluOpType.add)
                            nc.vector.tensor_scalar_mul(yw, yw, 0.25)
                            nc.vector.tensor_tensor(yw, yw, sk[:], AluOpType.add)
                            nc.vector.tensor_tensor(yw, yw, bias_sb[:],
                                                    AluOpType.add)
                        else:
                            for hh in range(4):
                                nc.vector.tensor_tensor(
                                    yw[:, hh * 16: (hh + 1) * 16],
                                    pwin[:, hh * 16: (hh + 1) * 16],
                                    rec[:, hh: hh + 1].to_broadcast([P, 16]),
                                    AluOpType.mult)
                            nc.vector.tensor_tensor(yw, yw, sk[:], AluOpType.add)
                            nc.vector.tensor_tensor(yw, yw, bias_sb[:],
                                                    AluOpType.add)
                            if relu:
                                nc.vector.tensor_scalar_max(yw, yw, 0.0)

            nc.sync.dma_start(
                y_out[:].rearrange("(w p) c -> p w c", p=P), y_sb[:])
    nc.compile()
    return nc


# ------------------------------------------------------------------ driver

_CACHE = {}
_DBG = []
_EXEC_NS = []


def _blockdiag(a):
    H, C = a.shape
    m = np.zeros((H * C, H), np.float32)
    for hh in range(H):
        m[hh * C: (hh + 1) * C, hh] = a[hh]
    return m


def _sent01():
    row = np.zeros((2, 64), np.float32)
    row[:, 32:36] = SENT_ALS
    return row.view(np.uint16).view(ml_dtypes.bfloat16)  # [2, 128]


def kernel(**inp):
    x = np.asarray(inp["x"], np.float32)
    ei = np.asarray(inp["edge_index"], np.int64)
    N, IN = x.shape
    E = ei.shape[1]

    loops = np.arange(N, dtype=np.int64)
    src = np.concatenate([ei[0], loops])
    dst = np.concatenate([ei[1], loops])

    pkey = ("plan", N, E, hash(ei.tobytes()))
    if pkey not in _CACHE:
        _CACHE[pkey] = build_plan(src, dst, N)
    shared, plans = _CACHE[pkey]
    Dp, D = shared["Dp"], shared["D"]
    Npad = _round_up(N, P)

    def prep01(Wv, a_s, a_d, cb, sW, sb, g, b, m, v):
        Wv, sW = np.asarray(Wv, np.float32), np.asarray(sW, np.float32)
        bns = (np.asarray(g) / np.sqrt(np.asarray(v) + EPS)).astype(np.float32)
        bnt = (np.asarray(b) - np.asarray(m) * bns).astype(np.float32)
        Waug = np.concatenate(
            [Wv * bns[None, :], Wv @ _blockdiag(np.asarray(a_s)),
             Wv @ _blockdiag(np.asarray(a_d))], 1)
        return (Waug, sW * bns[None, :], np.asarray(cb) * bns
                + np.asarray(sb) * bns + bnt, _sent01(), None)

    def prep2(Wv, a_s, a_d, cb, sW, sb):
        Wv = np.asarray(Wv, np.float32)
        a_s = np.asarray(a_s, np.float32)
        Waug = np.concatenate([Wv, Wv @ _blockdiag(np.asarray(a_d))], 1)
        hsent = np.zeros(256, np.float32)
        for hh in range(4):
            a = a_s[hh]
            hsent[hh * 64: (hh + 1) * 64] = SENT_ALS * a / (a * a).sum()
        sent = np.tile(hsent.astype(ml_dtypes.bfloat16), (2, 1))
        a2s_rep = np.tile(a_s.reshape(1, 1, 256).astype(ml_dtypes.bfloat16),
                          (P, 1, 1))
        return (Waug, np.asarray(sW, np.float32),
                np.asarray(cb) + np.asarray(sb), sent, a2s_rep)

    Ls = [
        prep01(inp["conv0_W"], inp["conv0_as"], inp["conv0_ad"], inp["conv0_b"],
               inp["skip0_W"], inp["skip0_b"], inp["bn0_g"], inp["bn0_b"],
               inp["bn0_m"], inp["bn0_v"]),
        prep01(inp["conv1_W"], inp["conv1_as"], inp["conv1_ad"], inp["conv1_b"],
               inp["skip1_W"], inp["skip1_b"], inp["bn1_g"], inp["bn1_b"],
               inp["bn1_m"], inp["bn1_v"]),
        prep2(inp["conv2_W"], inp["conv2_as"], inp["conv2_ad"], inp["conv2_b"],
              inp["skip2_W"], inp["skip2_b"]),
    ]

    rep = np.zeros((len(KLIST), 16, P), np.float32)
    for ki, k in enumerate(KLIST):
        for p in range(P):
            rep[ki, p // k, p] = 1.0
    rep_np = rep.reshape(len(KLIST) * 16, P)
    svar_np = SVAR_NP.astype(ml_dtypes.bfloat16)

    h = x
    for li in range(3):
        F = IN if li == 0 else 64
        OUTW = 64 if li < 2 else 256
        mean_heads = li == 2
        Waug, skipWf, biasv, sent, a2s_rep = Ls[li]
        lkey = ("nc", li, F, OUTW, N, E)
        if lkey not in _CACHE:
            _CACHE[lkey] = build_layer(
                shared, F, OUTW, relu=not mean_heads, mean_heads=mean_heads,
                tcap=64 if not mean_heads else 32,
                gb=8 if not mean_heads else 3)
        nck = _CACHE[lkey]

        hT_full = np.zeros((F, Npad), np.float32)
        hT_full[:, :N] = h.T
        base = {
            "hT": hT_full,
            "Waug": Waug.astype(np.float32),
            "skipW": skipWf.astype(np.float32),
            "biasR": np.tile(biasv.astype(np.float32), (P, 1)),
            "svar": svar_np,
            "rep": rep_np,
            "sent": np.asarray(sent, ml_dtypes.bfloat16),
        }
        if mean_heads:
            base["a2s"] = a2s_rep
        in_maps = []
        for c in range(NC):
            vmap = plans[c]["vmap"]
            hTow = np.zeros((F, Dp), np.float32)
            valid = vmap >= 0
            hTow[:, valid] = h[c * D + vmap[valid]].T
            in_maps.append(dict(base, hTow=hTow,
                                idx_lo=plans[c]["idx_lo"],
                                idx_hi=plans[c]["idx_hi"]))
        import time as _time
        _t0 = _time.time()
        res = run_bass_kernel_spmd(nck, in_maps, core_ids=list(range(NC)))
        if res.exec_time_ns:
            _EXEC_NS.append(res.exec_time_ns)
        print(f"  layer {li} run wall: {_time.time()-_t0:.1f}s", flush=True)
        hn = np.zeros((N, 64), np.float32)
        for c in range(NC):
            vmap = plans[c]["vmap"]
            valid = vmap >= 0
            hn[c * D + vmap[valid]] = res.results[c]["y"][valid]
        h = hn
        _DBG.append(h)
    return h



# revision 5
# speedup vs baseline: 1.7230x; 1.7230x over previous
"""GAT (3-layer, PyG-style) Trainium2 Bass kernel, 8-core dst-sharded. v2.

Self-contained: takes full inputs, shards internally, returns full output.

Design (v2, exact-CSR):
  - dst nodes sharded across 8 cores; per layer one SPMD launch.
  - dense phase: PE builds a DRAM gather table of 2-node pair rows
    (node payload: feats fp16 | al_s f32), plus per-dst al_d kept in SBUF.
  - edge phase: slots = edges sorted by dst (exact CSR, no K-bucketing).
    Window w (128 dsts) owns a fixed run of slabs (128 slots each); slab
    counts per window are padded to the max over cores so one SPMD program
    fits all cores.  Per slab:
      selT (pos->slot one-hot, DVE is_equal vs iota) broadcasts al_d to
      slots via a PE matmul; z = al_s + al_d + M (M = -100 static mask
      kills pad slots and the wrong node half), ex = exp(leaky(z)); the
      features of both halves are scaled by their ex, and sel (slot->pos
      one-hot) accumulates [sum ex*feat | sum ex] per dst window in PSUM.
    Drain divides by sum ex, adds skip matmul + bias (+BN fold, +ReLU;
    layer 2 means over heads).
  - src is indexed as pair rows (idx = src//2 < 32768 fits int16); the
    wrong half of each gathered pair row is annihilated by the M mask.
"""
import numpy as np
import ml_dtypes

import concourse.bacc as bacc
import concourse.mybir as mybir
import concourse.tile as tile
from concourse.alu_op_type import AluOpType
from concourse.bass_utils import run_bass_kernel_spmd

BF16 = mybir.dt.bfloat16
FP16 = mybir.dt.float16
F32 = mybir.dt.float32
I16 = mybir.dt.int16

NC = 8
P = 128
EPS = 1e-5
MPEN = -100.0
NOPOS = 1000.0


def _round_up(x, m):
    return (x + m - 1) // m * m


# ----------------------------------------------------------------- planning

def build_plan(src, dst, N):
    D = N // NC
    nwin = (D + P - 1) // P

    core = dst // D
    dloc = dst % D
    win = dloc // P

    # slabs per window: max over cores (uniform SPMD structure)
    wdeg = np.zeros((NC, nwin), np.int64)
    np.add.at(wdeg, (core, win), 1)
    spw = (wdeg.max(axis=0) + P - 1) // P  # [nwin]
    nslab = int(spw.sum())
    slab_win = np.repeat(np.arange(nwin), spw)  # [nslab]
    first_slab = {}
    last_slab = {}
    for i, w in enumerate(slab_win):
        first_slab.setdefault(int(w), i)
        last_slab[int(w)] = i
    wslab0 = np.zeros(nwin, np.int64)
    np.cumsum(spw[:-1], out=wslab0[1:])
    nslot = nslab * P

    shared = dict(N=N, D=D, nwin=nwin, spw=spw, nslab=nslab,
                  slab_win=slab_win, first_slab=first_slab,
                  last_slab=last_slab, nslot=nslot)

    plans = []
    for c in range(NC):
        em = core == c
        es = src[em]
        ed = dloc[em]
        o = np.argsort(ed, kind="stable")
        es, ed = es[o], ed[o]
        # slot arrays (padded)
        s_idx = np.zeros(nslot, np.int64)       # pair-row index
        s_par = np.zeros(nslot, np.int64)       # parity (which half)
        s_pos = np.full(nslot, -1, np.int64)    # dst pos in window, -1 = pad
        wstart = np.searchsorted(ed // P, np.arange(nwin), side="left")
        wend = np.searchsorted(ed // P, np.arange(nwin), side="right")
        for w in range(nwin):
            a, b = int(wstart[w]), int(wend[w])
            o0 = int(wslab0[w]) * P
            n = b - a
            s_idx[o0: o0 + n] = es[a:b] // 2
            s_par[o0: o0 + n] = es[a:b] % 2
            s_pos[o0: o0 + n] = ed[a:b] % P
        # wrapped idx [128, nslot//16]
        iw = s_idx.reshape(nslot // 16, 16).T.astype(np.int16)
        idx_w = np.tile(iw, (8, 1))
        # dstpos tiles
        posv = np.where(s_pos >= 0, s_pos, NOPOS).astype(np.float32).astype(ml_dtypes.bfloat16)
        posm = posv.reshape(nslab, P)                      # [slab, slot]
        dstposP = posm.T.copy()                            # [128 slot, nslab]
        dstposF = np.broadcast_to(
            posm[None, :, :], (P, nslab, P)).copy()
        # M masks [128 slot, nslab] f32
        real = (s_pos >= 0).reshape(nslab, P).T
        parE = (s_par == 0).reshape(nslab, P).T
        M_E = np.where(real & parE, 0.0, MPEN).astype(np.float32)
        M_O = np.where(real & ~parE, 0.0, MPEN).astype(np.float32)
        plans.append(dict(idx=idx_w, dstposP=dstposP, dstposF=dstposF,
                          M_E=M_E, M_O=M_O))
    return shared, plans


# ------------------------------------------------------------- kernel build

def build_layer(shared, F, L2, tcap, gb):
    """L2: concat=False layer (256-wide feats, mean over heads)."""
    N, D, nwin, nslab = shared["N"], shared["D"], shared["nwin"], shared["nslab"]
    slab_win = shared["slab_win"]
    first_slab, last_slab = shared["first_slab"], shared["last_slab"]
    nslot = shared["nslot"]

    OUTW = 256 if L2 else 64        # feat cols per node (fp16)
    NAW = OUTW + 4
    RWH = 384 if L2 else 128        # fp16 cols per node payload
    RW = 2 * RWH                    # fp16 cols per pair row
    NA = OUTW + 8                   # dense out: feats | als | ald
    Npad = _round_up(N, P)
    nchunk = Npad // P
    NPAIR = Npad // 2
    Dpad = nwin * P

    groups = []
    s0 = 0
    while s0 < nslab:
        groups.append((s0, min(s0 + tcap, nslab)))
        s0 += tcap

    nc = bacc.Bacc("TRN2", target_bir_lowering=False, debug=False)
    hT = nc.dram_tensor("hT", [F, Npad], F32, kind="ExternalInput")
    hTow = nc.dram_tensor("hTow", [F, Dpad], F32, kind="ExternalInput")
    Waug = nc.dram_tensor("Waug", [F, NA], F32, kind="ExternalInput")
    skipW = nc.dram_tensor("skipW", [F, 64], F32, kind="ExternalInput")
    biasR = nc.dram_tensor("biasR", [P, 64], F32, kind="ExternalInput")
    iotaF_in = nc.dram_tensor("iotaF", [P, P], BF16, kind="ExternalInput")
    iotaP_in = nc.dram_tensor("iotaP", [P, 1], BF16, kind="ExternalInput")
    dposP_in = nc.dram_tensor("dposP", [P, nslab], BF16, kind="ExternalInput")
    dposF_in = nc.dram_tensor("dposF", [P, nslab, P], BF16,
                              kind="ExternalInput")
    ME_in = nc.dram_tensor("M_E", [P, nslab], F32, kind="ExternalInput")
    MO_in = nc.dram_tensor("M_O", [P, nslab], F32, kind="ExternalInput")
    idx_in = nc.dram_tensor("idx", [P, nslot // 16], I16, kind="ExternalInput")

    table = nc.dram_tensor("table", [NPAIR, RW], BF16, kind="Internal")
    y_out = nc.dram_tensor("y", [Dpad, 64], F32, kind="ExternalOutput")

    with tile.TileContext(nc) as tc:
        with (
            tc.tile_pool(name="const", bufs=1) as cp,
            tc.tile_pool(name="ybuf", bufs=1) as yp,
        ):
            waug_sb = cp.tile([F, NA], F32)
            nc.sync.dma_start(waug_sb[:], Waug[:])
            skipw_sb = cp.tile([F, 64], F32)
            nc.sync.dma_start(skipw_sb[:], skipW[:])
            bias_sb = cp.tile([P, 64], F32)
            nc.sync.dma_start(bias_sb[:], biasR[:])
            iotaF = cp.tile([P, P], BF16)
            nc.sync.dma_start(iotaF[:], iotaF_in[:])
            iotaP = cp.tile([P, 1], BF16)
            nc.sync.dma_start(iotaP[:], iotaP_in[:])
            dposP = cp.tile([P, nslab], BF16)
            nc.sync.dma_start(dposP[:], dposP_in[:])
            ME_sb = cp.tile([P, nslab], F32)
            nc.sync.dma_start(ME_sb[:], ME_in[:])
            MO_sb = cp.tile([P, nslab], F32)
            nc.sync.dma_start(MO_sb[:], MO_in[:])
            idx_sb = cp.tile([P, nslot // 16], I16)
            nc.sync.dma_start(idx_sb[:], idx_in[:])
            hTow_sb = cp.tile([F, Dpad], F32)
            nc.sync.dma_start(hTow_sb[:], hTow[:])
            y_sb = yp.tile([P, nwin, 64], F32)
            aldw_sb = cp.tile([P, nwin, 4], F32)

            # ---------------- dense phase: build gather table + al_d
            with (
                tc.tile_pool(name="dstage", bufs=3) as dsp,
                tc.tile_pool(name="pdense", bufs=2, space="PSUM") as pd,
                tc.tile_pool(name="pal", bufs=2, space="PSUM") as pal,
            ):
                NAp = 128 if not L2 else 512  # bank-aligned per-chunk stride
                for g0 in range(0, nchunk, gb):
                    g1 = min(g0 + gb, nchunk)
                    ng = g1 - g0
                    stage = dsp.tile([F, gb * P], F32, tag="stage")
                    nc.sync.dma_start(stage[:, : ng * P], hT[:, g0 * P:g1 * P])
                    dps = pd.tile([P, gb * NAp], F32, space="PSUM", tag="dps")
                    for i in range(ng):
                        nc.tensor.matmul(
                            dps[:, i * NAp: i * NAp + NA],
                            stage[:, i * P: (i + 1) * P],
                            waug_sb[:],
                            start=True, stop=True,
                        )
                    tstage = dsp.tile([P, gb, RWH], BF16, tag="tstage")
                    dv = dps[:].rearrange("p (i w) -> p i w", w=NAp)
                    nc.vector.tensor_copy(tstage[:, :ng, 0:OUTW],
                                          dv[:, :ng, 0:OUTW])
                    tf32 = tstage[:].bitcast(F32)
                    nc.vector.tensor_copy(tf32[:, :ng, OUTW // 2: OUTW // 2 + 4],
                                          dv[:, :ng, OUTW: OUTW + 4])
                    for i in range(ng):
                        ch = g0 + i
                        nc.sync.dma_start(
                            table[ch * 64: (ch + 1) * 64, :].rearrange(
                                "r (h w) -> (r h) w", h=2),
                            tstage[:, i, :],
                        )
                # al_d for owned dsts
                for w in range(nwin):
                    aps = pal.tile([P, 4], F32, space="PSUM", tag="alw")
                    nc.tensor.matmul(
                        aps[:], hTow_sb[:, w * P: (w + 1) * P],
                        waug_sb[:, NA - 4: NA], start=True, stop=True)
                    nc.vector.tensor_copy(aldw_sb[:, w, :], aps[:])

            # ---------------- edge phase
            with (
                tc.tile_pool(name="gpool", bufs=2) as gp,
                tc.tile_pool(name="dfpool", bufs=2) as dfp,
                tc.tile_pool(name="spool", bufs=4) as ssp,
                tc.tile_pool(name="zpool", bufs=2) as zp,
                tc.tile_pool(name="pwin", bufs=3, space="PSUM") as pw,
                tc.tile_pool(name="palde", bufs=2, space="PSUM") as pa,
                tc.tile_pool(name="psk", bufs=2, space="PSUM") as pk,
            ):
                win_ps = {}
                for (s0, s1) in groups:
                    T = s1 - s0
                    gt = gp.tile([P, tcap, RW], BF16, tag="G")
                    nc.gpsimd.dma_gather(
                        gt[:, :T, :], table[:], idx_sb[:, s0 * 8: s1 * 8],
                        T * P, T * P, RW, single_packet=False)
                    gf = gt[:].bitcast(F32)

                    dposF = dfp.tile([P, tcap, P], BF16, tag="df")
                    nc.sync.dma_start(dposF[:, :T, :], dposF_in[:, s0:s1, :])

                    # al_d -> slots (selT matmuls), then z/ex per parity
                    alde = pa.tile([P, 512], F32, space="PSUM", tag="alde")
                    for t in range(T):
                        s = s0 + t
                        w = int(slab_win[s])
                        selT = ssp.tile([P, P], F32, tag="selT")
                        nc.vector.tensor_tensor(
                            selT[:], iotaP[:].to_broadcast([P, P]),
                            dposF[:, t, :], AluOpType.is_equal)
                        nc.tensor.matmul(
                            alde[:, t * 4: t * 4 + 4], selT[:],
                            aldw_sb[:, w, :], start=True, stop=True,
                            skip_group_check=True)
                    aldv = alde[:, : T * 4].rearrange("p (t h) -> p t h", h=4)

                    for par, off in ((0, 0), (1, RWH)):
                        z = zp.tile([P, tcap, 4], F32, tag=f"z{par}")
                        als = gf[:, :T, off // 2 + OUTW // 2:
                                 off // 2 + OUTW // 2 + 4]
                        nc.vector.tensor_tensor(z[:, :T, :], als, aldv[:, :T, :],
                                                AluOpType.add)
                        m_sb = ME_sb if par == 0 else MO_sb
                        nc.vector.tensor_tensor(
                            z[:, :T, :], z[:, :T, :],
                            m_sb[:, s0:s1].rearrange(
                                "p (t o) -> p t o", o=1).to_broadcast([P, T, 4]),
                            AluOpType.add)
                        nc.vector.scalar_tensor_tensor(
                            z[:, :T, :], z[:, :T, :], 0.2, z[:, :T, :],
                            AluOpType.mult, AluOpType.max)
                        nc.scalar.activation(
                            gt[:, :T, off + OUTW: off + OUTW + 4], z[:, :T, :],
                            mybir.ActivationFunctionType.Exp)
                        nh = 4
                        hw = OUTW // 4
                        for hh in range(nh):
                            nc.vector.tensor_tensor(
                                gt[:, :T, off + hh * hw: off + (hh + 1) * hw],
                                gt[:, :T, off + hh * hw: off + (hh + 1) * hw],
                                gt[:, :T, off + OUTW + hh: off + OUTW + hh + 1
                                   ].to_broadcast([P, T, hw]),
                                AluOpType.mult)

                    # window accumulation
                    for t in range(T):
                        s = s0 + t
                        w = int(slab_win[s])
                        sel = ssp.tile([P, P], BF16, tag="sel")
                        nc.vector.tensor_tensor(
                            sel[:],
                            dposP[:, s: s + 1].to_broadcast([P, P]),
                            iotaF[:], AluOpType.is_equal)
                        if w not in win_ps:
                            win_ps[w] = pw.tile([P, 512], F32, space="PSUM",
                                                tag="win", name=f"win{w}")
                        st = first_slab[w] == s
                        fin = last_slab[w] == s
                        nc.tensor.matmul(
                            win_ps[w][:, 0:NAW], sel[:], gt[:, t, 0:NAW],
                            start=st, stop=False, skip_group_check=True)
                        nc.tensor.matmul(
                            win_ps[w][:, 0:NAW], sel[:],
                            gt[:, t, RWH: RWH + NAW],
                            start=False, stop=fin, skip_group_check=True)
                        if not fin:
                            continue
                        # ---- drain window w
                        pwin = win_ps.pop(w)
                        sk = pk.tile([P, 512], F32, space="PSUM", tag="skps")
                        nc.tensor.matmul(
                            sk[:, 0:64], hTow_sb[:, w * P: (w + 1) * P],
                            skipw_sb[:], start=True, stop=True)
                        rec = ssp.tile([P, 4], F32, tag="rec")
                        nc.vector.reciprocal(rec[:], pwin[:, OUTW: OUTW + 4])
                        yw = y_sb[:, w, :]
                        if L2:
                            m_t = ssp.tile([P, 4, 64], F32, tag="mt")
                            for hh in range(4):
                                nc.vector.tensor_tensor(
                                    m_t[:, hh, :],
                                    pwin[:, hh * 64: (hh + 1) * 64],
                                    rec[:, hh: hh + 1].to_broadcast([P, 64]),
                                    AluOpType.mult)
                            nc.vector.tensor_tensor(yw, m_t[:, 0, :],
                                                    m_t[:, 1, :], AluOpType.add)
                            nc.vector.tensor_tensor(yw, yw, m_t[:, 2, :],
                                                    AluOpType.add)
                            nc.vector.tensor_tensor(yw, yw, m_t[:, 3, :],
                                                    AluOpType.add)
                            nc.vector.tensor_scalar_mul(yw, yw, 0.25)
                            nc.vector.tensor_tensor(yw, yw, sk[:, 0:64],
                                                    AluOpType.add)
                            nc.vector.tensor_tensor(yw, yw, bias_sb[:],
                                                    AluOpType.add)
                        else:
                            for hh in range(4):
                                nc.vector.tensor_tensor(
                                    yw[:, hh * 16: (hh + 1) * 16],
                                    pwin[:, hh * 16: (hh + 1) * 16],
                                    rec[:, hh: hh + 1].to_broadcast([P, 16]),
                                    AluOpType.mult)
                            nc.vector.tensor_tensor(yw, yw, sk[:, 0:64],
                                                    AluOpType.add)
                            nc.vector.tensor_tensor(yw, yw, bias_sb[:],
                                                    AluOpType.add)
                            nc.vector.tensor_scalar_max(yw, yw, 0.0)

            nc.sync.dma_start(
                y_out[:].rearrange("(w p) c -> p w c", p=P), y_sb[:])
    nc.compile()
    return nc


# ------------------------------------------------------------------ driver

_CACHE = {}
_EXEC_NS = []


def _blockdiag(a):
    H, C = a.shape
    m = np.zeros((H * C, H), np.float32)
    for hh in range(H):
        m[hh * C: (hh + 1) * C, hh] = a[hh]
    return m


def kernel(**inp):
    x = np.asarray(inp["x"], np.float32)
    ei = np.asarray(inp["edge_index"], np.int64)
    N, IN = x.shape
    E = ei.shape[1]

    loops = np.arange(N, dtype=np.int64)
    src = np.concatenate([ei[0], loops])
    dst = np.concatenate([ei[1], loops])

    pkey = ("plan", N, E, hash(ei.tobytes()))
    if pkey not in _CACHE:
        _CACHE[pkey] = build_plan(src, dst, N)
    shared, plans = _CACHE[pkey]
    D, nwin = shared["D"], shared["nwin"]
    Dpad = nwin * P
    Npad = _round_up(N, P)

    def prep01(Wv, a_s, a_d, cb, sW, sb, g, b, m, v):
        Wv, sW = np.asarray(Wv, np.float32), np.asarray(sW, np.float32)
        bns = (np.asarray(g) / np.sqrt(np.asarray(v) + EPS)).astype(np.float32)
        bnt = (np.asarray(b) - np.asarray(m) * bns).astype(np.float32)
        Waug = np.concatenate(
            [Wv * bns[None, :], Wv @ _blockdiag(np.asarray(a_s)),
             Wv @ _blockdiag(np.asarray(a_d))], 1)
        return (Waug, sW * bns[None, :],
                np.asarray(cb) * bns + np.asarray(sb) * bns + bnt)

    def prep2(Wv, a_s, a_d, cb, sW, sb):
        Wv = np.asarray(Wv, np.float32)
        Waug = np.concatenate(
            [Wv, Wv @ _blockdiag(np.asarray(a_s)),
             Wv @ _blockdiag(np.asarray(a_d))], 1)
        return (Waug, np.asarray(sW, np.float32),
                np.asarray(cb) + np.asarray(sb))

    Ls = [
        prep01(inp["conv0_W"], inp["conv0_as"], inp["conv0_ad"], inp["conv0_b"],
               inp["skip0_W"], inp["skip0_b"], inp["bn0_g"], inp["bn0_b"],
               inp["bn0_m"], inp["bn0_v"]),
        prep01(inp["conv1_W"], inp["conv1_as"], inp["conv1_ad"], inp["conv1_b"],
               inp["skip1_W"], inp["skip1_b"], inp["bn1_g"], inp["bn1_b"],
               inp["bn1_m"], inp["bn1_v"]),
        prep2(inp["conv2_W"], inp["conv2_as"], inp["conv2_ad"], inp["conv2_b"],
              inp["skip2_W"], inp["skip2_b"]),
    ]

    iotaF = np.tile(np.arange(P, dtype=np.float32), (P, 1)).astype(ml_dtypes.bfloat16)
    iotaP = np.arange(P, dtype=np.float32).reshape(P, 1).astype(ml_dtypes.bfloat16)

    h = x
    for li in range(3):
        F = IN if li == 0 else 64
        L2 = li == 2
        Waug, skipWf, biasv = Ls[li]
        lkey = ("nc", li, F, N, E)
        if lkey not in _CACHE:
            _CACHE[lkey] = build_layer(
                shared, F, L2, tcap=16 if L2 else 32, gb=2 if L2 else 4)
        nck = _CACHE[lkey]

        hT_full = np.zeros((F, Npad), np.float32)
        hT_full[:, :N] = h.T
        base = {
            "hT": hT_full,
            "Waug": Waug.astype(np.float32),
            "skipW": skipWf.astype(np.float32),
            "biasR": np.tile(biasv.astype(np.float32), (P, 1)),
            "iotaF": iotaF, "iotaP": iotaP,
        }
        in_maps = []
        for c in range(NC):
            pl = plans[c]
            hTow = np.zeros((F, Dpad), np.float32)
            hTow[:, :D] = h[c * D: (c + 1) * D].T
            in_maps.append(dict(base, hTow=hTow, idx=pl["idx"],
                                dposP=pl["dstposP"], dposF=pl["dstposF"],
                                M_E=pl["M_E"], M_O=pl["M_O"]))
        import time as _time
        _t0 = _time.time()
        res = run_bass_kernel_spmd(nck, in_maps, core_ids=list(range(NC)))
        if res.exec_time_ns:
            _EXEC_NS.append(res.exec_time_ns)
        print(f"  layer {li} run wall: {_time.time()-_t0:.1f}s", flush=True)
        hn = np.zeros((N, 64), np.float32)
        for c in range(NC):
            hn[c * D: (c + 1) * D] = res.results[c]["y"][:D]
        h = hn
    return h


# revision 6
# speedup vs baseline: 1.7470x; 1.0139x over previous
"""GAT (3-layer, PyG-style) Trainium2 Bass kernel, 8-core dst-sharded. v2.

Self-contained: takes full inputs, shards internally, returns full output.

Design (v2, exact-CSR):
  - dst nodes sharded across 8 cores; per layer one SPMD launch.
  - dense phase: PE builds a DRAM gather table of 2-node pair rows
    (node payload: feats fp16 | al_s f32), plus per-dst al_d kept in SBUF.
  - edge phase: slots = edges sorted by dst (exact CSR, no K-bucketing).
    Window w (128 dsts) owns a fixed run of slabs (128 slots each); slab
    counts per window are padded to the max over cores so one SPMD program
    fits all cores.  Per slab:
      selT (pos->slot one-hot, DVE is_equal vs iota) broadcasts al_d to
      slots via a PE matmul; z = al_s + al_d + M (M = -100 static mask
      kills pad slots and the wrong node half), ex = exp(leaky(z)); the
      features of both halves are scaled by their ex, and sel (slot->pos
      one-hot) accumulates [sum ex*feat | sum ex] per dst window in PSUM.
    Drain divides by sum ex, adds skip matmul + bias (+BN fold, +ReLU;
    layer 2 means over heads).
  - src is indexed as pair rows (idx = src//2 < 32768 fits int16); the
    wrong half of each gathered pair row is annihilated by the M mask.
"""
import numpy as np
import ml_dtypes

import concourse.bacc as bacc
import concourse.mybir as mybir
import concourse.tile as tile
from concourse.alu_op_type import AluOpType
from concourse.bass_utils import run_bass_kernel_spmd

BF16 = mybir.dt.bfloat16
FP16 = mybir.dt.float16
F32 = mybir.dt.float32
I16 = mybir.dt.int16

NC = 8
P = 128
EPS = 1e-5
MPEN = -100.0
NOPOS = 1000.0


def _round_up(x, m):
    return (x + m - 1) // m * m


# ----------------------------------------------------------------- planning

def build_plan(src, dst, N):
    D = N // NC
    nwin = (D + P - 1) // P

    core = dst // D
    dloc = dst % D
    win = dloc // P

    # slabs per window: max over cores (uniform SPMD structure)
    wdeg = np.zeros((NC, nwin), np.int64)
    np.add.at(wdeg, (core, win), 1)
    spw = (wdeg.max(axis=0) + P - 1) // P  # [nwin]
    nslab = int(spw.sum())
    slab_win = np.repeat(np.arange(nwin), spw)  # [nslab]
    first_slab = {}
    last_slab = {}
    for i, w in enumerate(slab_win):
        first_slab.setdefault(int(w), i)
        last_slab[int(w)] = i
    wslab0 = np.zeros(nwin, np.int64)
    np.cumsum(spw[:-1], out=wslab0[1:])
    nslot = nslab * P

    shared = dict(N=N, D=D, nwin=nwin, spw=spw, nslab=nslab,
                  slab_win=slab_win, first_slab=first_slab,
                  last_slab=last_slab, nslot=nslot)

    plans = []
    for c in range(NC):
        em = core == c
        es = src[em]
        ed = dloc[em]
        o = np.argsort(ed, kind="stable")
        es, ed = es[o], ed[o]
        # slot arrays (padded)
        s_idx = np.zeros(nslot, np.int64)       # pair-row index
        s_par = np.zeros(nslot, np.int64)       # parity (which half)
        s_pos = np.full(nslot, -1, np.int64)    # dst pos in window, -1 = pad
        wstart = np.searchsorted(ed // P, np.arange(nwin), side="left")
        wend = np.searchsorted(ed // P, np.arange(nwin), side="right")
        for w in range(nwin):
            a, b = int(wstart[w]), int(wend[w])
            o0 = int(wslab0[w]) * P
            n = b - a
            s_idx[o0: o0 + n] = es[a:b] // 2
            s_par[o0: o0 + n] = es[a:b] % 2
            s_pos[o0: o0 + n] = ed[a:b] % P
        # wrapped idx [128, nslot//16]
        iw = s_idx.reshape(nslot // 16, 16).T.astype(np.int16)
        idx_w = np.tile(iw, (8, 1))
        # dstpos tiles
        posv = np.where(s_pos >= 0, s_pos, NOPOS).astype(np.float32).astype(ml_dtypes.bfloat16)
        posm = posv.reshape(nslab, P)                      # [slab, slot]
        dstposP = posm.T.copy()                            # [128 slot, nslab]
        dstposF = np.broadcast_to(
            posm[None, :, :], (P, nslab, P)).copy()
        # M masks [128 slot, nslab] f32
        real = (s_pos >= 0).reshape(nslab, P).T
        parE = (s_par == 0).reshape(nslab, P).T
        M_E = np.where(real & parE, 0.0, MPEN).astype(np.float32)
        M_O = np.where(real & ~parE, 0.0, MPEN).astype(np.float32)
        plans.append(dict(idx=idx_w, dstposP=dstposP, dstposF=dstposF,
                          M_E=M_E, M_O=M_O))
    return shared, plans


# ------------------------------------------------------------- kernel build

def build_layer(shared, F, L2, tcap, gb):
    """L2: concat=False layer (256-wide feats, mean over heads)."""
    N, D, nwin, nslab = shared["N"], shared["D"], shared["nwin"], shared["nslab"]
    slab_win = shared["slab_win"]
    first_slab, last_slab = shared["first_slab"], shared["last_slab"]
    nslot = shared["nslot"]

    OUTW = 256 if L2 else 64        # feat cols per node (fp16)
    NAW = OUTW + 4
    RWH = 384 if L2 else 128        # fp16 cols per node payload
    RW = 2 * RWH                    # fp16 cols per pair row
    NA = OUTW + 8                   # dense out: feats | als | ald
    Npad = _round_up(N, P)
    nchunk = Npad // P
    NPAIR = Npad // 2
    Dpad = nwin * P

    groups = []
    s0 = 0
    while s0 < nslab:
        groups.append((s0, min(s0 + tcap, nslab)))
        s0 += tcap

    nc = bacc.Bacc("TRN2", target_bir_lowering=False, debug=False)
    hT_hi = nc.dram_tensor("hT_hi", [F, Npad], BF16, kind="ExternalInput")
    hT_lo = nc.dram_tensor("hT_lo", [F, Npad], BF16, kind="ExternalInput")
    hTow = nc.dram_tensor("hTow", [F, Dpad], F32, kind="ExternalInput")
    Waug = nc.dram_tensor("Waug", [F, NA], F32, kind="ExternalInput")
    Waug_hi = nc.dram_tensor("Waug_hi", [F, NA], BF16, kind="ExternalInput")
    Waug_lo = nc.dram_tensor("Waug_lo", [F, NA], BF16, kind="ExternalInput")
    skipW = nc.dram_tensor("skipW", [F, 64], F32, kind="ExternalInput")
    biasR = nc.dram_tensor("biasR", [P, 64], F32, kind="ExternalInput")
    iotaF_in = nc.dram_tensor("iotaF", [P, P], BF16, kind="ExternalInput")
    iotaP_in = nc.dram_tensor("iotaP", [P, 1], BF16, kind="ExternalInput")
    dposP_in = nc.dram_tensor("dposP", [P, nslab], BF16, kind="ExternalInput")
    dposF_in = nc.dram_tensor("dposF", [P, nslab, P], BF16,
                              kind="ExternalInput")
    ME_in = nc.dram_tensor("M_E", [P, nslab], F32, kind="ExternalInput")
    MO_in = nc.dram_tensor("M_O", [P, nslab], F32, kind="ExternalInput")
    idx_in = nc.dram_tensor("idx", [P, nslot // 16], I16, kind="ExternalInput")

    table = nc.dram_tensor("table", [NPAIR, RW], BF16, kind="Internal")
    y_out = nc.dram_tensor("y", [Dpad, 64], F32, kind="ExternalOutput")

    with tile.TileContext(nc) as tc:
        with (
            tc.tile_pool(name="const", bufs=1) as cp,
            tc.tile_pool(name="ybuf", bufs=1) as yp,
        ):
            waug_sb = cp.tile([F, NA], F32)
            nc.sync.dma_start(waug_sb[:], Waug[:])
            waugh_sb = cp.tile([F, NA], BF16)
            nc.sync.dma_start(waugh_sb[:], Waug_hi[:])
            waugl_sb = cp.tile([F, NA], BF16)
            nc.sync.dma_start(waugl_sb[:], Waug_lo[:])
            skipw_sb = cp.tile([F, 64], F32)
            nc.sync.dma_start(skipw_sb[:], skipW[:])
            bias_sb = cp.tile([P, 64], F32)
            nc.sync.dma_start(bias_sb[:], biasR[:])
            iotaF = cp.tile([P, P], BF16)
            nc.sync.dma_start(iotaF[:], iotaF_in[:])
            iotaP = cp.tile([P, 1], BF16)
            nc.sync.dma_start(iotaP[:], iotaP_in[:])
            dposP = cp.tile([P, nslab], BF16)
            nc.sync.dma_start(dposP[:], dposP_in[:])
            ME_sb = cp.tile([P, nslab], F32)
            nc.sync.dma_start(ME_sb[:], ME_in[:])
            MO_sb = cp.tile([P, nslab], F32)
            nc.sync.dma_start(MO_sb[:], MO_in[:])
            idx_sb = cp.tile([P, nslot // 16], I16)
            nc.sync.dma_start(idx_sb[:], idx_in[:])
            hTow_sb = cp.tile([F, Dpad], F32)
            nc.sync.dma_start(hTow_sb[:], hTow[:])
            y_sb = yp.tile([P, nwin, 64], F32)
            aldh_sb = cp.tile([P, nwin, 4], BF16)
            aldl_sb = cp.tile([P, nwin, 4], BF16)

            # ---------------- dense phase: build gather table + al_d
            with (
                tc.tile_pool(name="dstage", bufs=3) as dsp,
                tc.tile_pool(name="pdense", bufs=2, space="PSUM") as pd,
                tc.tile_pool(name="pal", bufs=2, space="PSUM") as pal,
            ):
                NAp = 128 if not L2 else 512  # bank-aligned per-chunk stride
                for g0 in range(0, nchunk, gb):
                    g1 = min(g0 + gb, nchunk)
                    ng = g1 - g0
                    stg_h = dsp.tile([F, gb * P], BF16, tag="stg_h")
                    nc.sync.dma_start(stg_h[:, : ng * P], hT_hi[:, g0 * P:g1 * P])
                    stg_l = dsp.tile([F, gb * P], BF16, tag="stg_l")
                    nc.sync.dma_start(stg_l[:, : ng * P], hT_lo[:, g0 * P:g1 * P])
                    dps = pd.tile([P, gb * NAp], F32, space="PSUM", tag="dps")
                    for i in range(ng):
                        o = i * NAp
                        nc.tensor.matmul(
                            dps[:, o: o + NA], stg_h[:, i * P: (i + 1) * P],
                            waugh_sb[:], start=True, stop=False,
                            skip_group_check=True)
                        nc.tensor.matmul(
                            dps[:, o: o + NA], stg_h[:, i * P: (i + 1) * P],
                            waugl_sb[:], start=False, stop=False,
                            skip_group_check=True)
                        nc.tensor.matmul(
                            dps[:, o: o + NA], stg_l[:, i * P: (i + 1) * P],
                            waugh_sb[:], start=False, stop=True,
                            skip_group_check=True)
                    tstage = dsp.tile([P, gb, RWH], BF16, tag="tstage")
                    dv = dps[:].rearrange("p (i w) -> p i w", w=NAp)
                    nc.vector.tensor_copy(tstage[:, :ng, 0:OUTW],
                                          dv[:, :ng, 0:OUTW])
                    tf32 = tstage[:].bitcast(F32)
                    nc.vector.tensor_copy(tf32[:, :ng, OUTW // 2: OUTW // 2 + 4],
                                          dv[:, :ng, OUTW: OUTW + 4])
                    for i in range(ng):
                        ch = g0 + i
                        nc.sync.dma_start(
                            table[ch * 64: (ch + 1) * 64, :].rearrange(
                                "r (h w) -> (r h) w", h=2),
                            tstage[:, i, :],
                        )
                # al_d for owned dsts
                for w in range(nwin):
                    aps = pal.tile([P, 4], F32, space="PSUM", tag="alw")
                    nc.tensor.matmul(
                        aps[:], hTow_sb[:, w * P: (w + 1) * P],
                        waug_sb[:, NA - 4: NA], start=True, stop=True)
                    nc.vector.tensor_copy(aldh_sb[:, w, :], aps[:])
                    alr = dsp.tile([P, 4], F32, tag="alr")
                    nc.vector.tensor_tensor(alr[:], aps[:], aldh_sb[:, w, :],
                                            AluOpType.subtract)
                    nc.vector.tensor_copy(aldl_sb[:, w, :], alr[:])

            # ---------------- edge phase
            with (
                tc.tile_pool(name="gpool", bufs=3) as gp,
                tc.tile_pool(name="dfpool", bufs=2) as dfp,
                tc.tile_pool(name="spool", bufs=4) as ssp,
                tc.tile_pool(name="zpool", bufs=2) as zp,
                tc.tile_pool(name="pwin", bufs=3, space="PSUM") as pw,
                tc.tile_pool(name="palde", bufs=2, space="PSUM") as pa,
                tc.tile_pool(name="psk", bufs=2, space="PSUM") as pk,
            ):
                win_ps = {}
                for (s0, s1) in groups:
                    T = s1 - s0
                    gt = gp.tile([P, tcap, RW], BF16, tag="G")
                    nc.gpsimd.dma_gather(
                        gt[:, :T, :], table[:], idx_sb[:, s0 * 8: s1 * 8],
                        T * P, T * P, RW, single_packet=False)
                    gf = gt[:].bitcast(F32)

                    dposF = dfp.tile([P, tcap, P], BF16, tag="df")
                    nc.sync.dma_start(dposF[:, :T, :], dposF_in[:, s0:s1, :])

                    # batched one-hot generation for the whole group
                    selT_g = ssp.tile([P, tcap, P], BF16, tag="selT")
                    nc.vector.tensor_tensor(
                        selT_g[:, :T, :],
                        iotaP[:].rearrange("p (o f) -> p o f", o=1
                                           ).to_broadcast([P, T, P]),
                        dposF[:, :T, :], AluOpType.is_equal)
                    sel_g = ssp.tile([P, tcap, P], BF16, tag="sel")
                    nc.vector.tensor_tensor(
                        sel_g[:, :T, :],
                        dposP[:, s0:s1].rearrange("p (t o) -> p t o", o=1
                                                  ).to_broadcast([P, T, P]),
                        iotaF[:].rearrange("p (o f) -> p o f", o=1
                                           ).to_broadcast([P, T, P]),
                        AluOpType.is_equal)

                    # al_d -> slots (selT matmuls), then z/ex per parity
                    alde = pa.tile([P, 512], F32, space="PSUM", tag="alde")
                    for t in range(T):
                        s = s0 + t
                        w = int(slab_win[s])
                        nc.tensor.matmul(
                            alde[:, t * 4: t * 4 + 4], selT_g[:, t, :],
                            aldh_sb[:, w, :], start=True, stop=False,
                            skip_group_check=True)
                        nc.tensor.matmul(
                            alde[:, t * 4: t * 4 + 4], selT_g[:, t, :],
                            aldl_sb[:, w, :], start=False, stop=True,
                            skip_group_check=True)
                    aldv = alde[:, : T * 4].rearrange("p (t h) -> p t h", h=4)

                    for par, off in ((0, 0), (1, RWH)):
                        z = zp.tile([P, tcap, 4], F32, tag=f"z{par}")
                        als = gf[:, :T, off // 2 + OUTW // 2:
                                 off // 2 + OUTW // 2 + 4]
                        nc.vector.tensor_tensor(z[:, :T, :], als, aldv[:, :T, :],
                                                AluOpType.add)
                        m_sb = ME_sb if par == 0 else MO_sb
                        nc.vector.tensor_tensor(
                            z[:, :T, :], z[:, :T, :],
                            m_sb[:, s0:s1].rearrange(
                                "p (t o) -> p t o", o=1).to_broadcast([P, T, 4]),
                            AluOpType.add)
                        nc.vector.scalar_tensor_tensor(
                            z[:, :T, :], z[:, :T, :], 0.2, z[:, :T, :],
                            AluOpType.mult, AluOpType.max)
                        nc.scalar.activation(
                            gt[:, :T, off + OUTW: off + OUTW + 4], z[:, :T, :],
                            mybir.ActivationFunctionType.Exp)
                        nh = 4
                        hw = OUTW // 4
                        for hh in range(nh):
                            nc.vector.tensor_tensor(
                                gt[:, :T, off + hh * hw: off + (hh + 1) * hw],
                                gt[:, :T, off + hh * hw: off + (hh + 1) * hw],
                                gt[:, :T, off + OUTW + hh: off + OUTW + hh + 1
                                   ].to_broadcast([P, T, hw]),
                                AluOpType.mult)

                    # window accumulation
                    for t in range(T):
                        s = s0 + t
                        w = int(slab_win[s])
                        sel = sel_g[:, t]
                        if w not in win_ps:
                            win_ps[w] = pw.tile([P, 512], F32, space="PSUM",
                                                tag="win", name=f"win{w}")
                        st = first_slab[w] == s
                        fin = last_slab[w] == s
                        nc.tensor.matmul(
                            win_ps[w][:, 0:NAW], sel, gt[:, t, 0:NAW],
                            start=st, stop=False, skip_group_check=True)
                        nc.tensor.matmul(
                            win_ps[w][:, 0:NAW], sel,
                            gt[:, t, RWH: RWH + NAW],
                            start=False, stop=fin, skip_group_check=True)
                        if not fin:
                            continue
                        # ---- drain window w
                        pwin = win_ps.pop(w)
                        sk = pk.tile([P, 512], F32, space="PSUM", tag="skps")
                        nc.tensor.matmul(
                            sk[:, 0:64], hTow_sb[:, w * P: (w + 1) * P],
                            skipw_sb[:], start=True, stop=True)
                        rec = ssp.tile([P, 4], F32, tag="rec")
                        nc.vector.reciprocal(rec[:], pwin[:, OUTW: OUTW + 4])
                        yw = y_sb[:, w, :]
                        if L2:
                            m_t = ssp.tile([P, 4, 64], F32, tag="mt")
                            for hh in range(4):
                                nc.vector.tensor_tensor(
                                    m_t[:, hh, :],
                                    pwin[:, hh * 64: (hh + 1) * 64],
                                    rec[:, hh: hh + 1].to_broadcast([P, 64]),
                                    AluOpType.mult)
                            nc.vector.tensor_tensor(yw, m_t[:, 0, :],
                                                    m_t[:, 1, :], AluOpType.add)
                            nc.vector.tensor_tensor(yw, yw, m_t[:, 2, :],
                                                    AluOpType.add)
                            nc.vector.tensor_tensor(yw, yw, m_t[:, 3, :],
                                                    AluOpType.add)
                            nc.vector.tensor_scalar_mul(yw, yw, 0.25)
                            nc.vector.tensor_tensor(yw, yw, sk[:, 0:64],
                                                    AluOpType.add)
                            nc.vector.tensor_tensor(yw, yw, bias_sb[:],
                                                    AluOpType.add)
                        else:
                            for hh in range(4):
                                nc.vector.tensor_tensor(
                                    yw[:, hh * 16: (hh + 1) * 16],
                                    pwin[:, hh * 16: (hh + 1) * 16],
                                    rec[:, hh: hh + 1].to_broadcast([P, 16]),
                                    AluOpType.mult)
                            nc.vector.tensor_tensor(yw, yw, sk[:, 0:64],
                                                    AluOpType.add)
                            nc.vector.tensor_tensor(yw, yw, bias_sb[:],
                                                    AluOpType.add)
                            nc.vector.tensor_scalar_max(yw, yw, 0.0)

            nc.sync.dma_start(
                y_out[:].rearrange("(w p) c -> p w c", p=P), y_sb[:])
    nc.compile()
    return nc


# ------------------------------------------------------------------ driver

_CACHE = {}
_EXEC_NS = []


def _blockdiag(a):
    H, C = a.shape
    m = np.zeros((H * C, H), np.float32)
    for hh in range(H):
        m[hh * C: (hh + 1) * C, hh] = a[hh]
    return m


def kernel(**inp):
    x = np.asarray(inp["x"], np.float32)
    ei = np.asarray(inp["edge_index"], np.int64)
    N, IN = x.shape
    E = ei.shape[1]

    loops = np.arange(N, dtype=np.int64)
    src = np.concatenate([ei[0], loops])
    dst = np.concatenate([ei[1], loops])

    pkey = ("plan", N, E, hash(ei.tobytes()))
    if pkey not in _CACHE:
        _CACHE[pkey] = build_plan(src, dst, N)
    shared, plans = _CACHE[pkey]
    D, nwin = shared["D"], shared["nwin"]
    Dpad = nwin * P
    Npad = _round_up(N, P)

    def prep01(Wv, a_s, a_d, cb, sW, sb, g, b, m, v):
        Wv, sW = np.asarray(Wv, np.float32), np.asarray(sW, np.float32)
        bns = (np.asarray(g) / np.sqrt(np.asarray(v) + EPS)).astype(np.float32)
        bnt = (np.asarray(b) - np.asarray(m) * bns).astype(np.float32)
        Waug = np.concatenate(
            [Wv * bns[None, :], Wv @ _blockdiag(np.asarray(a_s)),
             Wv @ _blockdiag(np.asarray(a_d))], 1)
        return (Waug, sW * bns[None, :],
                np.asarray(cb) * bns + np.asarray(sb) * bns + bnt)

    def prep2(Wv, a_s, a_d, cb, sW, sb):
        Wv = np.asarray(Wv, np.float32)
        Waug = np.concatenate(
            [Wv, Wv @ _blockdiag(np.asarray(a_s)),
             Wv @ _blockdiag(np.asarray(a_d))], 1)
        return (Waug, np.asarray(sW, np.float32),
                np.asarray(cb) + np.asarray(sb))

    Ls = [
        prep01(inp["conv0_W"], inp["conv0_as"], inp["conv0_ad"], inp["conv0_b"],
               inp["skip0_W"], inp["skip0_b"], inp["bn0_g"], inp["bn0_b"],
               inp["bn0_m"], inp["bn0_v"]),
        prep01(inp["conv1_W"], inp["conv1_as"], inp["conv1_ad"], inp["conv1_b"],
               inp["skip1_W"], inp["skip1_b"], inp["bn1_g"], inp["bn1_b"],
               inp["bn1_m"], inp["bn1_v"]),
        prep2(inp["conv2_W"], inp["conv2_as"], inp["conv2_ad"], inp["conv2_b"],
              inp["skip2_W"], inp["skip2_b"]),
    ]

    iotaF = np.tile(np.arange(P, dtype=np.float32), (P, 1)).astype(ml_dtypes.bfloat16)
    iotaP = np.arange(P, dtype=np.float32).reshape(P, 1).astype(ml_dtypes.bfloat16)

    h = x
    for li in range(3):
        F = IN if li == 0 else 64
        L2 = li == 2
        Waug, skipWf, biasv = Ls[li]
        lkey = ("nc", li, F, N, E)
        if lkey not in _CACHE:
            _CACHE[lkey] = build_layer(
                shared, F, L2, tcap=16 if L2 else 32, gb=2 if L2 else 4)
        nck = _CACHE[lkey]

        hT_full = np.zeros((F, Npad), np.float32)
        hT_full[:, :N] = h.T
        hT_hi = hT_full.astype(ml_dtypes.bfloat16)
        hT_lo = (hT_full - hT_hi.astype(np.float32)).astype(ml_dtypes.bfloat16)
        Waug32 = Waug.astype(np.float32)
        Waug_hi = Waug32.astype(ml_dtypes.bfloat16)
        Waug_lo = (Waug32 - Waug_hi.astype(np.float32)).astype(ml_dtypes.bfloat16)
        base = {
            "hT_hi": hT_hi, "hT_lo": hT_lo,
            "Waug": Waug32, "Waug_hi": Waug_hi, "Waug_lo": Waug_lo,
            "skipW": skipWf.astype(np.float32),
            "biasR": np.tile(biasv.astype(np.float32), (P, 1)),
            "iotaF": iotaF, "iotaP": iotaP,
        }
        in_maps = []
        for c in range(NC):
            pl = plans[c]
            hTow = np.zeros((F, Dpad), np.float32)
            hTow[:, :D] = h[c * D: (c + 1) * D].T
            in_maps.append(dict(base, hTow=hTow, idx=pl["idx"],
                                dposP=pl["dstposP"], dposF=pl["dstposF"],
                                M_E=pl["M_E"], M_O=pl["M_O"]))
        import time as _time
        _t0 = _time.time()
        res = run_bass_kernel_spmd(nck, in_maps, core_ids=list(range(NC)))
        if res.exec_time_ns:
            _EXEC_NS.append(res.exec_time_ns)
        print(f"  layer {li} run wall: {_time.time()-_t0:.1f}s", flush=True)
        hn = np.zeros((N, 64), np.float32)
        for c in range(NC):
            hn[c * D: (c + 1) * D] = res.results[c]["y"][:D]
        h = hn
    return h


# revision 8
# speedup vs baseline: 1.8067x; 1.0342x over previous
"""GAT (3-layer, PyG-style) Trainium2 Bass kernel, 8-core dst-sharded. v2.

Self-contained: takes full inputs, shards internally, returns full output.

Design (v2, exact-CSR):
  - dst nodes sharded across 8 cores; per layer one SPMD launch.
  - dense phase: PE builds a DRAM gather table of 2-node pair rows
    (node payload: feats fp16 | al_s f32), plus per-dst al_d kept in SBUF.
  - edge phase: slots = edges sorted by dst (exact CSR, no K-bucketing).
    Window w (128 dsts) owns a fixed run of slabs (128 slots each); slab
    counts per window are padded to the max over cores so one SPMD program
    fits all cores.  Per slab:
      selT (pos->slot one-hot, DVE is_equal vs iota) broadcasts al_d to
      slots via a PE matmul; z = al_s + al_d + M (M = -100 static mask
      kills pad slots and the wrong node half), ex = exp(leaky(z)); the
      features of both halves are scaled by their ex, and sel (slot->pos
      one-hot) accumulates [sum ex*feat | sum ex] per dst window in PSUM.
    Drain divides by sum ex, adds skip matmul + bias (+BN fold, +ReLU;
    layer 2 means over heads).
  - src is indexed as pair rows (idx = src//2 < 32768 fits int16); the
    wrong half of each gathered pair row is annihilated by the M mask.
"""
import numpy as np
import ml_dtypes

import concourse.bacc as bacc
import concourse.mybir as mybir
import concourse.tile as tile
from concourse.alu_op_type import AluOpType
from concourse.bass_utils import run_bass_kernel_spmd

BF16 = mybir.dt.bfloat16
FP16 = mybir.dt.float16
F32 = mybir.dt.float32
I16 = mybir.dt.int16

NC = 8
P = 128
EPS = 1e-5
MPEN = -100.0
NOPOS = 1000.0


def _round_up(x, m):
    return (x + m - 1) // m * m


# ----------------------------------------------------------------- planning

def build_plan(src, dst, N):
    D = N // NC
    nwin = (D + P - 1) // P

    core = dst // D
    dloc = dst % D
    win = dloc // P

    # slabs per window: max over cores (uniform SPMD structure)
    wdeg = np.zeros((NC, nwin), np.int64)
    np.add.at(wdeg, (core, win), 1)
    spw = (wdeg.max(axis=0) + P - 1) // P  # [nwin]
    nslab = int(spw.sum())
    slab_win = np.repeat(np.arange(nwin), spw)  # [nslab]
    first_slab = {}
    last_slab = {}
    for i, w in enumerate(slab_win):
        first_slab.setdefault(int(w), i)
        last_slab[int(w)] = i
    wslab0 = np.zeros(nwin, np.int64)
    np.cumsum(spw[:-1], out=wslab0[1:])
    nslot = nslab * P

    shared = dict(N=N, D=D, nwin=nwin, spw=spw, nslab=nslab,
                  slab_win=slab_win, first_slab=first_slab,
                  last_slab=last_slab, nslot=nslot)

    plans = []
    for c in range(NC):
        em = core == c
        es = src[em]
        ed = dloc[em]
        o = np.argsort(ed, kind="stable")
        es, ed = es[o], ed[o]
        # slot arrays (padded)
        s_idx = np.zeros(nslot, np.int64)       # pair-row index
        s_par = np.zeros(nslot, np.int64)       # parity (which half)
        s_pos = np.full(nslot, -1, np.int64)    # dst pos in window, -1 = pad
        wstart = np.searchsorted(ed // P, np.arange(nwin), side="left")
        wend = np.searchsorted(ed // P, np.arange(nwin), side="right")
        for w in range(nwin):
            a, b = int(wstart[w]), int(wend[w])
            o0 = int(wslab0[w]) * P
            n = b - a
            s_idx[o0: o0 + n] = es[a:b] // 2
            s_par[o0: o0 + n] = es[a:b] % 2
            s_pos[o0: o0 + n] = ed[a:b] % P
        # wrapped idx [128, nslot//16]
        iw = s_idx.reshape(nslot // 16, 16).T.astype(np.int16)
        idx_w = np.tile(iw, (8, 1))
        # dstpos tiles
        posv = np.where(s_pos >= 0, s_pos, NOPOS).astype(np.float32).astype(ml_dtypes.bfloat16)
        posm = posv.reshape(nslab, P)                      # [slab, slot]
        dstposP = posm.T.copy()                            # [128 slot, nslab]
        dstposF = np.broadcast_to(
            posm[None, :, :], (P, nslab, P)).copy()
        # M masks [128 slot, nslab] f32
        real = (s_pos >= 0).reshape(nslab, P).T
        parE = (s_par == 0).reshape(nslab, P).T
        M_E = np.where(real & parE, 0.0, MPEN).astype(np.float32)
        M_O = np.where(real & ~parE, 0.0, MPEN).astype(np.float32)
        plans.append(dict(idx=idx_w, dstposP=dstposP, dstposF=dstposF,
                          M_E=M_E, M_O=M_O))
    return shared, plans


# ------------------------------------------------------------- kernel build

def build_layer(shared, F, L2, tcap, gb):
    """L2: concat=False layer (256-wide feats, mean over heads)."""
    N, D, nwin, nslab = shared["N"], shared["D"], shared["nwin"], shared["nslab"]
    slab_win = shared["slab_win"]
    first_slab, last_slab = shared["first_slab"], shared["last_slab"]
    nslot = shared["nslot"]

    OUTW = 256 if L2 else 64        # feat cols per node (fp16)
    NAW = OUTW + 4
    RWH = 384 if L2 else 128        # fp16 cols per node payload
    RW = 2 * RWH                    # fp16 cols per pair row
    NA = OUTW + 8                   # dense out: feats | als | ald
    Npad = _round_up(N, P)
    nchunk = Npad // P
    NPAIR = Npad // 2
    Dpad = nwin * P

    groups = []
    s0 = 0
    while s0 < nslab:
        groups.append((s0, min(s0 + tcap, nslab)))
        s0 += tcap

    nc = bacc.Bacc("TRN2", target_bir_lowering=False, debug=False)
    hT_hi = nc.dram_tensor("hT_hi", [F, Npad], BF16, kind="ExternalInput")
    hT_lo = nc.dram_tensor("hT_lo", [F, Npad], BF16, kind="ExternalInput")
    hTow = nc.dram_tensor("hTow", [F, Dpad], F32, kind="ExternalInput")
    Waug = nc.dram_tensor("Waug", [F, NA], F32, kind="ExternalInput")
    Waug_hi = nc.dram_tensor("Waug_hi", [F, NA], BF16, kind="ExternalInput")
    Waug_lo = nc.dram_tensor("Waug_lo", [F, NA], BF16, kind="ExternalInput")
    skipW = nc.dram_tensor("skipW", [F, 64], F32, kind="ExternalInput")
    biasR = nc.dram_tensor("biasR", [P, 64], F32, kind="ExternalInput")
    iotaF_in = nc.dram_tensor("iotaF", [P, P], BF16, kind="ExternalInput")
    iotaP_in = nc.dram_tensor("iotaP", [P, 1], BF16, kind="ExternalInput")
    dposP_in = nc.dram_tensor("dposP", [P, nslab], BF16, kind="ExternalInput")
    dposF_in = nc.dram_tensor("dposF", [P, nslab, P], BF16,
                              kind="ExternalInput")
    ME_in = nc.dram_tensor("M_E", [P, nslab], F32, kind="ExternalInput")
    MO_in = nc.dram_tensor("M_O", [P, nslab], F32, kind="ExternalInput")
    idx_in = nc.dram_tensor("idx", [P, nslot // 16], I16, kind="ExternalInput")

    table = nc.dram_tensor("table", [NPAIR, RW], BF16, kind="Internal")
    y_out = nc.dram_tensor("y", [Dpad, 64], F32, kind="ExternalOutput")

    with tile.TileContext(nc) as tc:
        with (
            tc.tile_pool(name="const", bufs=1) as cp,
            tc.tile_pool(name="ybuf", bufs=1) as yp,
        ):
            waug_sb = cp.tile([F, NA], F32)
            nc.sync.dma_start(waug_sb[:], Waug[:])
            waugh_sb = cp.tile([F, NA], BF16)
            nc.sync.dma_start(waugh_sb[:], Waug_hi[:])
            waugl_sb = cp.tile([F, NA], BF16)
            nc.sync.dma_start(waugl_sb[:], Waug_lo[:])
            skipw_sb = cp.tile([F, 64], F32)
            nc.sync.dma_start(skipw_sb[:], skipW[:])
            bias_sb = cp.tile([P, 64], F32)
            nc.sync.dma_start(bias_sb[:], biasR[:])
            iotaF = cp.tile([P, P], BF16)
            nc.sync.dma_start(iotaF[:], iotaF_in[:])
            iotaP = cp.tile([P, 1], BF16)
            nc.sync.dma_start(iotaP[:], iotaP_in[:])
            dposP = cp.tile([P, nslab], BF16)
            nc.sync.dma_start(dposP[:], dposP_in[:])
            ME_sb = cp.tile([P, nslab], F32)
            nc.sync.dma_start(ME_sb[:], ME_in[:])
            MO_sb = cp.tile([P, nslab], F32)
            nc.sync.dma_start(MO_sb[:], MO_in[:])
            idx_sb = cp.tile([P, nslot // 16], I16)
            nc.sync.dma_start(idx_sb[:], idx_in[:])
            hTow_sb = cp.tile([F, Dpad], F32)
            nc.scalar.dma_start(hTow_sb[:], hTow[:])
            y_sb = yp.tile([P, nwin, 64], F32)
            aldh_sb = cp.tile([P, nwin, 4], BF16)
            aldl_sb = cp.tile([P, nwin, 4], BF16)

            # ---------------- al_d for owned dsts
            with (
                tc.tile_pool(name="alstage", bufs=2) as asp,
                tc.tile_pool(name="pal", bufs=2, space="PSUM") as pal,
            ):
                for w in range(nwin):
                    aps = pal.tile([P, 4], F32, space="PSUM", tag="alw")
                    nc.tensor.matmul(
                        aps[:], hTow_sb[:, w * P: (w + 1) * P],
                        waug_sb[:, NA - 4: NA], start=True, stop=True)
                    nc.vector.tensor_copy(aldh_sb[:, w, :], aps[:])
                    alr = asp.tile([P, 4], F32, tag="alr")
                    nc.vector.tensor_tensor(alr[:], aps[:], aldh_sb[:, w, :],
                                            AluOpType.subtract)
                    nc.vector.tensor_copy(aldl_sb[:, w, :], alr[:])

            # ---------------- dense phase: build gather table
            with (
                tc.tile_pool(name="dstage", bufs=3) as dsp,
                tc.tile_pool(name="pdense", bufs=2, space="PSUM") as pd,
            ):
                NAp = 128 if not L2 else 512  # bank-aligned per-chunk stride
                for g0 in range(0, nchunk, gb):
                    g1 = min(g0 + gb, nchunk)
                    ng = g1 - g0
                    stg_h = dsp.tile([F, gb * P], BF16, tag="stg_h")
                    nc.sync.dma_start(stg_h[:, : ng * P], hT_hi[:, g0 * P:g1 * P])
                    stg_l = dsp.tile([F, gb * P], BF16, tag="stg_l")
                    nc.scalar.dma_start(stg_l[:, : ng * P], hT_lo[:, g0 * P:g1 * P])
                    dps = pd.tile([P, gb * NAp], F32, space="PSUM", tag="dps")
                    for i in range(ng):
                        o = i * NAp
                        nc.tensor.matmul(
                            dps[:, o: o + NA], stg_h[:, i * P: (i + 1) * P],
                            waugh_sb[:], start=True, stop=False,
                            skip_group_check=True)
                        nc.tensor.matmul(
                            dps[:, o: o + NA], stg_h[:, i * P: (i + 1) * P],
                            waugl_sb[:], start=False, stop=False,
                            skip_group_check=True)
                        nc.tensor.matmul(
                            dps[:, o: o + NA], stg_l[:, i * P: (i + 1) * P],
                            waugh_sb[:], start=False, stop=True,
                            skip_group_check=True)
                    tstage = dsp.tile([P, gb, RWH], BF16, tag="tstage")
                    dv = dps[:].rearrange("p (i w) -> p i w", w=NAp)
                    nc.vector.tensor_copy(tstage[:, :ng, 0:OUTW],
                                          dv[:, :ng, 0:OUTW])
                    tf32 = tstage[:].bitcast(F32)
                    nc.vector.tensor_copy(tf32[:, :ng, OUTW // 2: OUTW // 2 + 4],
                                          dv[:, :ng, OUTW: OUTW + 4])
                    for i in range(ng):
                        ch = g0 + i
                        nc.scalar.dma_start(
                            table[ch * 64: (ch + 1) * 64, :].rearrange(
                                "r (h w) -> (r h) w", h=2),
                            tstage[:, i, :],
                        )

            # ---------------- edge phase
            with (
                tc.tile_pool(name="gpool", bufs=3) as gp,
                tc.tile_pool(name="dfpool", bufs=2) as dfp,
                tc.tile_pool(name="spool", bufs=4) as ssp,
                tc.tile_pool(name="zpool", bufs=2) as zp,
                tc.tile_pool(name="pwin", bufs=3, space="PSUM") as pw,
                tc.tile_pool(name="palde", bufs=2, space="PSUM") as pa,
                tc.tile_pool(name="psk", bufs=2, space="PSUM") as pk,
            ):
                win_ps = {}
                for (s0, s1) in groups:
                    T = s1 - s0
                    gt = gp.tile([P, tcap, RW], BF16, tag="G")
                    nc.gpsimd.dma_gather(
                        gt[:, :T, :], table[:], idx_sb[:, s0 * 8: s1 * 8],
                        T * P, T * P, RW, single_packet=False)
                    gf = gt[:].bitcast(F32)

                    dposF = dfp.tile([P, tcap, P], BF16, tag="df")
                    nc.sync.dma_start(dposF[:, :T, :], dposF_in[:, s0:s1, :])

                    # batched one-hot generation for the whole group
                    selT_g = ssp.tile([P, tcap, P], BF16, tag="selT")
                    nc.vector.tensor_tensor(
                        selT_g[:, :T, :],
                        iotaP[:].rearrange("p (o f) -> p o f", o=1
                                           ).to_broadcast([P, T, P]),
                        dposF[:, :T, :], AluOpType.is_equal)
                    sel_g = ssp.tile([P, tcap, P], BF16, tag="sel")
                    nc.vector.tensor_tensor(
                        sel_g[:, :T, :],
                        dposP[:, s0:s1].rearrange("p (t o) -> p t o", o=1
                                                  ).to_broadcast([P, T, P]),
                        iotaF[:].rearrange("p (o f) -> p o f", o=1
                                           ).to_broadcast([P, T, P]),
                        AluOpType.is_equal)

                    # al_d -> slots (selT matmuls), then z/ex per parity
                    alde = pa.tile([P, 512], F32, space="PSUM", tag="alde")
                    for t in range(T):
                        s = s0 + t
                        w = int(slab_win[s])
                        nc.tensor.matmul(
                            alde[:, t * 4: t * 4 + 4], selT_g[:, t, :],
                            aldh_sb[:, w, :], start=True, stop=False,
                            skip_group_check=True)
                        nc.tensor.matmul(
                            alde[:, t * 4: t * 4 + 4], selT_g[:, t, :],
                            aldl_sb[:, w, :], start=False, stop=True,
                            skip_group_check=True)
                    aldv = alde[:, : T * 4].rearrange("p (t h) -> p t h", h=4)

                    for par, off in ((0, 0), (1, RWH)):
                        z = zp.tile([P, tcap, 4], F32, tag=f"z{par}")
                        als = gf[:, :T, off // 2 + OUTW // 2:
                                 off // 2 + OUTW // 2 + 4]
                        nc.vector.tensor_tensor(z[:, :T, :], als, aldv[:, :T, :],
                                                AluOpType.add)
                        m_sb = ME_sb if par == 0 else MO_sb
                        nc.vector.tensor_tensor(
                            z[:, :T, :], z[:, :T, :],
                            m_sb[:, s0:s1].rearrange(
                                "p (t o) -> p t o", o=1).to_broadcast([P, T, 4]),
                            AluOpType.add)
                        nc.vector.scalar_tensor_tensor(
                            z[:, :T, :], z[:, :T, :], 0.2, z[:, :T, :],
                            AluOpType.mult, AluOpType.max)
                        nc.scalar.activation(
                            gt[:, :T, off + OUTW: off + OUTW + 4], z[:, :T, :],
                            mybir.ActivationFunctionType.Exp)
                        nh = 4
                        hw = OUTW // 4
                        for hh in range(nh):
                            nc.vector.tensor_tensor(
                                gt[:, :T, off + hh * hw: off + (hh + 1) * hw],
                                gt[:, :T, off + hh * hw: off + (hh + 1) * hw],
                                gt[:, :T, off + OUTW + hh: off + OUTW + hh + 1
                                   ].to_broadcast([P, T, hw]),
                                AluOpType.mult)

                    # window accumulation
                    for t in range(T):
                        s = s0 + t
                        w = int(slab_win[s])
                        sel = sel_g[:, t]
                        if w not in win_ps:
                            win_ps[w] = pw.tile([P, 512], F32, space="PSUM",
                                                tag="win", name=f"win{w}")
                        st = first_slab[w] == s
                        fin = last_slab[w] == s
                        nc.tensor.matmul(
                            win_ps[w][:, 0:NAW], sel, gt[:, t, 0:NAW],
                            start=st, stop=False, skip_group_check=True)
                        nc.tensor.matmul(
                            win_ps[w][:, 0:NAW], sel,
                            gt[:, t, RWH: RWH + NAW],
                            start=False, stop=fin, skip_group_check=True)
                        if not fin:
                            continue
                        # ---- drain window w
                        pwin = win_ps.pop(w)
                        sk = pk.tile([P, 512], F32, space="PSUM", tag="skps")
                        nc.tensor.matmul(
                            sk[:, 0:64], hTow_sb[:, w * P: (w + 1) * P],
                            skipw_sb[:], start=True, stop=True)
                        rec = ssp.tile([P, 4], F32, tag="rec")
                        nc.vector.reciprocal(rec[:], pwin[:, OUTW: OUTW + 4])
                        yw = y_sb[:, w, :]
                        if L2:
                            m_t = ssp.tile([P, 4, 64], F32, tag="mt")
                            for hh in range(4):
                                nc.vector.tensor_tensor(
                                    m_t[:, hh, :],
                                    pwin[:, hh * 64: (hh + 1) * 64],
                                    rec[:, hh: hh + 1].to_broadcast([P, 64]),
                                    AluOpType.mult)
                            nc.vector.tensor_tensor(yw, m_t[:, 0, :],
                                                    m_t[:, 1, :], AluOpType.add)
                            nc.vector.tensor_tensor(yw, yw, m_t[:, 2, :],
                                                    AluOpType.add)
                            nc.vector.tensor_tensor(yw, yw, m_t[:, 3, :],
                                                    AluOpType.add)
                            nc.vector.tensor_scalar_mul(yw, yw, 0.25)
                            nc.vector.tensor_tensor(yw, yw, sk[:, 0:64],
                                                    AluOpType.add)
                            nc.vector.tensor_tensor(yw, yw, bias_sb[:],
                                                    AluOpType.add)
                        else:
                            for hh in range(4):
                                nc.vector.tensor_tensor(
                                    yw[:, hh * 16: (hh + 1) * 16],
                                    pwin[:, hh * 16: (hh + 1) * 16],
                                    rec[:, hh: hh + 1].to_broadcast([P, 16]),
                                    AluOpType.mult)
                            nc.vector.tensor_tensor(yw, yw, sk[:, 0:64],
                                                    AluOpType.add)
                            nc.vector.tensor_tensor(yw, yw, bias_sb[:],
                                                    AluOpType.add)
                            nc.vector.tensor_scalar_max(yw, yw, 0.0)

            nc.sync.dma_start(
                y_out[:].rearrange("(w p) c -> p w c", p=P), y_sb[:])
    nc.compile()
    return nc


# ------------------------------------------------------------------ driver

_CACHE = {}
_EXEC_NS = []


def _blockdiag(a):
    H, C = a.shape
    m = np.zeros((H * C, H), np.float32)
    for hh in range(H):
        m[hh * C: (hh + 1) * C, hh] = a[hh]
    return m


def kernel(**inp):
    x = np.asarray(inp["x"], np.float32)
    ei = np.asarray(inp["edge_index"], np.int64)
    N, IN = x.shape
    E = ei.shape[1]

    loops = np.arange(N, dtype=np.int64)
    src = np.concatenate([ei[0], loops])
    dst = np.concatenate([ei[1], loops])

    pkey = ("plan", N, E, hash(ei.tobytes()))
    if pkey not in _CACHE:
        _CACHE[pkey] = build_plan(src, dst, N)
    shared, plans = _CACHE[pkey]
    D, nwin = shared["D"], shared["nwin"]
    Dpad = nwin * P
    Npad = _round_up(N, P)

    def prep01(Wv, a_s, a_d, cb, sW, sb, g, b, m, v):
        Wv, sW = np.asarray(Wv, np.float32), np.asarray(sW, np.float32)
        bns = (np.asarray(g) / np.sqrt(np.asarray(v) + EPS)).astype(np.float32)
        bnt = (np.asarray(b) - np.asarray(m) * bns).astype(np.float32)
        Waug = np.concatenate(
            [Wv * bns[None, :], Wv @ _blockdiag(np.asarray(a_s)),
             Wv @ _blockdiag(np.asarray(a_d))], 1)
        return (Waug, sW * bns[None, :],
                np.asarray(cb) * bns + np.asarray(sb) * bns + bnt)

    def prep2(Wv, a_s, a_d, cb, sW, sb):
        Wv = np.asarray(Wv, np.float32)
        Waug = np.concatenate(
            [Wv, Wv @ _blockdiag(np.asarray(a_s)),
             Wv @ _blockdiag(np.asarray(a_d))], 1)
        return (Waug, np.asarray(sW, np.float32),
                np.asarray(cb) + np.asarray(sb))

    Ls = [
        prep01(inp["conv0_W"], inp["conv0_as"], inp["conv0_ad"], inp["conv0_b"],
               inp["skip0_W"], inp["skip0_b"], inp["bn0_g"], inp["bn0_b"],
               inp["bn0_m"], inp["bn0_v"]),
        prep01(inp["conv1_W"], inp["conv1_as"], inp["conv1_ad"], inp["conv1_b"],
               inp["skip1_W"], inp["skip1_b"], inp["bn1_g"], inp["bn1_b"],
               inp["bn1_m"], inp["bn1_v"]),
        prep2(inp["conv2_W"], inp["conv2_as"], inp["conv2_ad"], inp["conv2_b"],
              inp["skip2_W"], inp["skip2_b"]),
    ]

    iotaF = np.tile(np.arange(P, dtype=np.float32), (P, 1)).astype(ml_dtypes.bfloat16)
    iotaP = np.arange(P, dtype=np.float32).reshape(P, 1).astype(ml_dtypes.bfloat16)

    h = x
    for li in range(3):
        F = IN if li == 0 else 64
        L2 = li == 2
        Waug, skipWf, biasv = Ls[li]
        lkey = ("nc", li, F, N, E)
        if lkey not in _CACHE:
            _CACHE[lkey] = build_layer(
                shared, F, L2, tcap=16 if L2 else 32, gb=4 if L2 else 8)
        nck = _CACHE[lkey]

        hT_full = np.zeros((F, Npad), np.float32)
        hT_full[:, :N] = h.T
        hT_hi = hT_full.astype(ml_dtypes.bfloat16)
        hT_lo = (hT_full - hT_hi.astype(np.float32)).astype(ml_dtypes.bfloat16)
        Waug32 = Waug.astype(np.float32)
        Waug_hi = Waug32.astype(ml_dtypes.bfloat16)
        Waug_lo = (Waug32 - Waug_hi.astype(np.float32)).astype(ml_dtypes.bfloat16)
        base = {
            "hT_hi": hT_hi, "hT_lo": hT_lo,
            "Waug": Waug32, "Waug_hi": Waug_hi, "Waug_lo": Waug_lo,
            "skipW": skipWf.astype(np.float32),
            "biasR": np.tile(biasv.astype(np.float32), (P, 1)),
            "iotaF": iotaF, "iotaP": iotaP,
        }
        in_maps = []
        for c in range(NC):
            pl = plans[c]
            hTow = np.zeros((F, Dpad), np.float32)
            hTow[:, :D] = h[c * D: (c + 1) * D].T
            in_maps.append(dict(base, hTow=hTow, idx=pl["idx"],
                                dposP=pl["dstposP"], dposF=pl["dstposF"],
                                M_E=pl["M_E"], M_O=pl["M_O"]))
        import time as _time
        _t0 = _time.time()
        res = run_bass_kernel_spmd(nck, in_maps, core_ids=list(range(NC)))
        if res.exec_time_ns:
            _EXEC_NS.append(res.exec_time_ns)
        print(f"  layer {li} run wall: {_time.time()-_t0:.1f}s", flush=True)
        hn = np.zeros((N, 64), np.float32)
        for c in range(NC):
            hn[c * D: (c + 1) * D] = res.results[c]["y"][:D]
        h = hn
    return h


# revision 10
# speedup vs baseline: 2.0376x; 1.1278x over previous
"""GAT (3-layer, PyG-style) Trainium2 Bass kernel, 8-core dst-sharded. v2.

Self-contained: takes full inputs, shards internally, returns full output.

Design (v2, exact-CSR):
  - dst nodes sharded across 8 cores; per layer one SPMD launch.
  - dense phase: PE builds a DRAM gather table of 2-node pair rows
    (node payload: feats fp16 | al_s f32), plus per-dst al_d kept in SBUF.
  - edge phase: slots = edges sorted by dst (exact CSR, no K-bucketing).
    Window w (128 dsts) owns a fixed run of slabs (128 slots each); slab
    counts per window are padded to the max over cores so one SPMD program
    fits all cores.  Per slab:
      selT (pos->slot one-hot, DVE is_equal vs iota) broadcasts al_d to
      slots via a PE matmul; z = al_s + al_d + M (M = -100 static mask
      kills pad slots and the wrong node half), ex = exp(leaky(z)); the
      features of both halves are scaled by their ex, and sel (slot->pos
      one-hot) accumulates [sum ex*feat | sum ex] per dst window in PSUM.
    Drain divides by sum ex, adds skip matmul + bias (+BN fold, +ReLU;
    layer 2 means over heads).
  - src is indexed as pair rows (idx = src//2 < 32768 fits int16); the
    wrong half of each gathered pair row is annihilated by the M mask.
"""
import numpy as np
import ml_dtypes

import concourse.bacc as bacc
import concourse.mybir as mybir
import concourse.tile as tile
from concourse.alu_op_type import AluOpType
from concourse.bass_utils import run_bass_kernel_spmd

BF16 = mybir.dt.bfloat16
FP16 = mybir.dt.float16
F32 = mybir.dt.float32
I16 = mybir.dt.int16

NC = 8
P = 128
EPS = 1e-5
MPEN = -100.0
NOPOS = 1000.0


def _round_up(x, m):
    return (x + m - 1) // m * m


# ----------------------------------------------------------------- planning

def build_plan(src, dst, N):
    D = N // NC
    nwin = (D + P - 1) // P
    Npad = _round_up(N, 2 * P)
    nchunk = Npad // P
    C2 = nchunk // 2

    core = dst // D
    dloc = dst % D
    win = dloc // P

    # slabs per window: max over cores (uniform SPMD structure)
    wdeg = np.zeros((NC, nwin), np.int64)
    np.add.at(wdeg, (core, win), 1)
    spw = (wdeg.max(axis=0) + P - 1) // P  # [nwin]
    nslab = int(spw.sum())
    slab_win = np.repeat(np.arange(nwin), spw)  # [nslab]
    first_slab = {}
    last_slab = {}
    for i, w in enumerate(slab_win):
        first_slab.setdefault(int(w), i)
        last_slab[int(w)] = i
    wslab0 = np.zeros(nwin, np.int64)
    np.cumsum(spw[:-1], out=wslab0[1:])
    nslot = nslab * P

    shared = dict(N=N, D=D, nwin=nwin, spw=spw, nslab=nslab,
                  slab_win=slab_win, first_slab=first_slab,
                  last_slab=last_slab, nslot=nslot)

    plans = []
    for c in range(NC):
        em = core == c
        es = src[em]
        ed = dloc[em]
        o = np.argsort(ed, kind="stable")
        es, ed = es[o], ed[o]
        # slot arrays (padded)
        s_idx = np.zeros(nslot, np.int64)       # pair-row index
        s_par = np.zeros(nslot, np.int64)       # parity (which half)
        s_pos = np.full(nslot, -1, np.int64)    # dst pos in window, -1 = pad
        wstart = np.searchsorted(ed // P, np.arange(nwin), side="left")
        wend = np.searchsorted(ed // P, np.arange(nwin), side="right")
        for w in range(nwin):
            a, b = int(wstart[w]), int(wend[w])
            o0 = int(wslab0[w]) * P
            n = b - a
            s_idx[o0: o0 + n] = (es[a:b] % P) * C2 + (es[a:b] // P) // 2
            s_par[o0: o0 + n] = (es[a:b] // P) % 2
            s_pos[o0: o0 + n] = ed[a:b] % P
        # wrapped idx [128, nslot//16]
        iw = s_idx.reshape(nslot // 16, 16).T.astype(np.int16)
        idx_w = np.tile(iw, (8, 1))
        # dstpos tiles
        posv = np.where(s_pos >= 0, s_pos, NOPOS).astype(np.float32).astype(ml_dtypes.bfloat16)
        posm = posv.reshape(nslab, P)                      # [slab, slot]
        dstposP = posm.T.copy()                            # [128 slot, nslab]
        dstposF = np.broadcast_to(
            posm[None, :, :], (P, nslab, P)).copy()
        # M masks [128 slot, nslab] f32
        real = (s_pos >= 0).reshape(nslab, P).T
        parE = (s_par == 0).reshape(nslab, P).T
        M_E = np.where(real & parE, 0.0, MPEN).astype(np.float32)
        M_O = np.where(real & ~parE, 0.0, MPEN).astype(np.float32)
        plans.append(dict(idx=idx_w, dstposP=dstposP, dstposF=dstposF,
                          M_E=M_E, M_O=M_O))
    return shared, plans


# ------------------------------------------------------------- kernel build

def build_layer(shared, F, L2, tcap, gb):
    """L2: concat=False layer (256-wide feats, mean over heads)."""
    N, D, nwin, nslab = shared["N"], shared["D"], shared["nwin"], shared["nslab"]
    slab_win = shared["slab_win"]
    first_slab, last_slab = shared["first_slab"], shared["last_slab"]
    nslot = shared["nslot"]

    OUTW = 256 if L2 else 64        # feat cols per node (fp16)
    NAW = OUTW + 4
    RWH = 384 if L2 else 128        # fp16 cols per node payload
    RW = 2 * RWH                    # fp16 cols per pair row
    NA = OUTW + 8                   # dense out: feats | als | ald
    Npad = _round_up(N, 2 * P)
    nchunk = Npad // P
    C2 = nchunk // 2
    NPAIR = Npad // 2
    Dpad = nwin * P

    groups = []
    s0 = 0
    while s0 < nslab:
        groups.append((s0, min(s0 + tcap, nslab)))
        s0 += tcap

    nc = bacc.Bacc("TRN2", target_bir_lowering=False, debug=False)
    hT_hi = nc.dram_tensor("hT_hi", [F, Npad], BF16, kind="ExternalInput")
    hT_lo = nc.dram_tensor("hT_lo", [F, Npad], BF16, kind="ExternalInput")
    hTow = nc.dram_tensor("hTow", [F, Dpad], F32, kind="ExternalInput")
    Waug = nc.dram_tensor("Waug", [F, NA], F32, kind="ExternalInput")
    Waug_hi = nc.dram_tensor("Waug_hi", [F, NA], BF16, kind="ExternalInput")
    Waug_lo = nc.dram_tensor("Waug_lo", [F, NA], BF16, kind="ExternalInput")
    skipW = nc.dram_tensor("skipW", [F, 64], F32, kind="ExternalInput")
    biasR = nc.dram_tensor("biasR", [P, 64], F32, kind="ExternalInput")
    iotaF_in = nc.dram_tensor("iotaF", [P, P], BF16, kind="ExternalInput")
    iotaP_in = nc.dram_tensor("iotaP", [P, 1], BF16, kind="ExternalInput")
    dposP_in = nc.dram_tensor("dposP", [P, nslab], BF16, kind="ExternalInput")
    dposF_in = nc.dram_tensor("dposF", [P, nslab, P], BF16,
                              kind="ExternalInput")
    ME_in = nc.dram_tensor("M_E", [P, nslab], F32, kind="ExternalInput")
    MO_in = nc.dram_tensor("M_O", [P, nslab], F32, kind="ExternalInput")
    idx_in = nc.dram_tensor("idx", [P, nslot // 16], I16, kind="ExternalInput")

    table = nc.dram_tensor("table", [NPAIR, RW], BF16, kind="Internal")
    y_out = nc.dram_tensor("y", [Dpad, 64], F32, kind="ExternalOutput")

    with tile.TileContext(nc) as tc:
        with (
            tc.tile_pool(name="const", bufs=1) as cp,
            tc.tile_pool(name="ybuf", bufs=1) as yp,
        ):
            waug_sb = cp.tile([F, NA], F32)
            nc.sync.dma_start(waug_sb[:], Waug[:])
            waugh_sb = cp.tile([F, NA], BF16)
            nc.sync.dma_start(waugh_sb[:], Waug_hi[:])
            waugl_sb = cp.tile([F, NA], BF16)
            nc.sync.dma_start(waugl_sb[:], Waug_lo[:])
            skipw_sb = cp.tile([F, 64], F32)
            nc.sync.dma_start(skipw_sb[:], skipW[:])
            bias_sb = cp.tile([P, 64], F32)
            nc.sync.dma_start(bias_sb[:], biasR[:])
            iotaF = cp.tile([P, P], BF16)
            nc.sync.dma_start(iotaF[:], iotaF_in[:])
            iotaP = cp.tile([P, 1], BF16)
            nc.sync.dma_start(iotaP[:], iotaP_in[:])
            dposP = cp.tile([P, nslab], BF16)
            nc.sync.dma_start(dposP[:], dposP_in[:])
            ME_sb = cp.tile([P, nslab], F32)
            nc.sync.dma_start(ME_sb[:], ME_in[:])
            MO_sb = cp.tile([P, nslab], F32)
            nc.sync.dma_start(MO_sb[:], MO_in[:])
            idx_sb = cp.tile([P, nslot // 16], I16)
            nc.sync.dma_start(idx_sb[:], idx_in[:])
            hTow_sb = cp.tile([F, Dpad], F32)
            nc.scalar.dma_start(hTow_sb[:], hTow[:])
            y_sb = yp.tile([P, nwin, 64], F32)
            aldh_sb = cp.tile([P, nwin, 4], BF16)
            aldl_sb = cp.tile([P, nwin, 4], BF16)

            # ---------------- al_d for owned dsts
            with (
                tc.tile_pool(name="alstage", bufs=2) as asp,
                tc.tile_pool(name="pal", bufs=2, space="PSUM") as pal,
            ):
                for w in range(nwin):
                    aps = pal.tile([P, 4], F32, space="PSUM", tag="alw")
                    nc.tensor.matmul(
                        aps[:], hTow_sb[:, w * P: (w + 1) * P],
                        waug_sb[:, NA - 4: NA], start=True, stop=True)
                    nc.vector.tensor_copy(aldh_sb[:, w, :], aps[:])
                    alr = asp.tile([P, 4], F32, tag="alr")
                    nc.vector.tensor_tensor(alr[:], aps[:], aldh_sb[:, w, :],
                                            AluOpType.subtract)
                    nc.vector.tensor_copy(aldl_sb[:, w, :], alr[:])

            # ---------------- dense phase: build gather table
            with (
                tc.tile_pool(name="dstage", bufs=3) as dsp,
                tc.tile_pool(name="pdense", bufs=2, space="PSUM") as pd,
            ):
                NAp = 128 if not L2 else 512  # bank-aligned per-chunk stride
                for g0 in range(0, nchunk, gb):
                    g1 = min(g0 + gb, nchunk)
                    ng = g1 - g0
                    stg_h = dsp.tile([F, gb * P], BF16, tag="stg_h")
                    nc.sync.dma_start(stg_h[:, : ng * P], hT_hi[:, g0 * P:g1 * P])
                    stg_l = dsp.tile([F, gb * P], BF16, tag="stg_l")
                    nc.scalar.dma_start(stg_l[:, : ng * P], hT_lo[:, g0 * P:g1 * P])
                    dps = pd.tile([P, gb * NAp], F32, space="PSUM", tag="dps")
                    for i in range(ng):
                        o = i * NAp
                        nc.tensor.matmul(
                            dps[:, o: o + NA], stg_h[:, i * P: (i + 1) * P],
                            waugh_sb[:], start=True, stop=False,
                            skip_group_check=True)
                        nc.tensor.matmul(
                            dps[:, o: o + NA], stg_h[:, i * P: (i + 1) * P],
                            waugl_sb[:], start=False, stop=False,
                            skip_group_check=True)
                        nc.tensor.matmul(
                            dps[:, o: o + NA], stg_l[:, i * P: (i + 1) * P],
                            waugh_sb[:], start=False, stop=True,
                            skip_group_check=True)
                    tstage = dsp.tile([P, gb, RWH], BF16, tag="tstage")
                    dv = dps[:].rearrange("p (i w) -> p i w", w=NAp)
                    nc.vector.tensor_copy(tstage[:, :ng, 0:OUTW],
                                          dv[:, :ng, 0:OUTW])
                    tf32 = tstage[:].bitcast(F32)
                    nc.vector.tensor_copy(tf32[:, :ng, OUTW // 2: OUTW // 2 + 4],
                                          dv[:, :ng, OUTW: OUTW + 4])
                    nc.scalar.dma_start(
                        table[:].rearrange(
                            "(p c2) (t w) -> p (c2 t) w", p=P, t=2
                        )[:, g0:g1, :],
                        tstage[:, :ng, :],
                    )

            # ---------------- edge phase
            with (
                tc.tile_pool(name="gpool", bufs=3) as gp,
                tc.tile_pool(name="dfpool", bufs=2) as dfp,
                tc.tile_pool(name="spool", bufs=4) as ssp,
                tc.tile_pool(name="zpool", bufs=2) as zp,
                tc.tile_pool(name="pwin", bufs=3, space="PSUM") as pw,
                tc.tile_pool(name="palde", bufs=2, space="PSUM") as pa,
                tc.tile_pool(name="psk", bufs=2, space="PSUM") as pk,
            ):
                win_ps = {}
                for (s0, s1) in groups:
                    T = s1 - s0
                    gt = gp.tile([P, tcap, RW], BF16, tag="G")
                    nc.gpsimd.dma_gather(
                        gt[:, :T, :], table[:], idx_sb[:, s0 * 8: s1 * 8],
                        T * P, T * P, RW, single_packet=False)
                    gf = gt[:].bitcast(F32)

                    dposF = dfp.tile([P, tcap, P], BF16, tag="df")
                    nc.sync.dma_start(dposF[:, :T, :], dposF_in[:, s0:s1, :])

                    # batched one-hot generation for the whole group
                    selT_g = ssp.tile([P, tcap, P], BF16, tag="selT")
                    nc.vector.tensor_tensor(
                        selT_g[:, :T, :],
                        iotaP[:].rearrange("p (o f) -> p o f", o=1
                                           ).to_broadcast([P, T, P]),
                        dposF[:, :T, :], AluOpType.is_equal)
                    sel_g = ssp.tile([P, tcap, P], BF16, tag="sel")
                    nc.vector.tensor_tensor(
                        sel_g[:, :T, :],
                        dposP[:, s0:s1].rearrange("p (t o) -> p t o", o=1
                                                  ).to_broadcast([P, T, P]),
                        iotaF[:].rearrange("p (o f) -> p o f", o=1
                                           ).to_broadcast([P, T, P]),
                        AluOpType.is_equal)

                    # al_d -> slots (selT matmuls), then z/ex per parity
                    alde = pa.tile([P, 512], F32, space="PSUM", tag="alde")
                    for t in range(T):
                        s = s0 + t
                        w = int(slab_win[s])
                        nc.tensor.matmul(
                            alde[:, t * 4: t * 4 + 4], selT_g[:, t, :],
                            aldh_sb[:, w, :], start=True, stop=False,
                            skip_group_check=True)
                        nc.tensor.matmul(
                            alde[:, t * 4: t * 4 + 4], selT_g[:, t, :],
                            aldl_sb[:, w, :], start=False, stop=True,
                            skip_group_check=True)
                    aldv = alde[:, : T * 4].rearrange("p (t h) -> p t h", h=4)

                    for par, off in ((0, 0), (1, RWH)):
                        z = zp.tile([P, tcap, 4], F32, tag=f"z{par}")
                        als = gf[:, :T, off // 2 + OUTW // 2:
                                 off // 2 + OUTW // 2 + 4]
                        nc.vector.tensor_tensor(z[:, :T, :], als, aldv[:, :T, :],
                                                AluOpType.add)
                        m_sb = ME_sb if par == 0 else MO_sb
                        nc.vector.tensor_tensor(
                            z[:, :T, :], z[:, :T, :],
                            m_sb[:, s0:s1].rearrange(
                                "p (t o) -> p t o", o=1).to_broadcast([P, T, 4]),
                            AluOpType.add)
                        nc.vector.scalar_tensor_tensor(
                            z[:, :T, :], z[:, :T, :], 0.2, z[:, :T, :],
                            AluOpType.mult, AluOpType.max)
                        nc.scalar.activation(
                            gt[:, :T, off + OUTW: off + OUTW + 4], z[:, :T, :],
                            mybir.ActivationFunctionType.Exp)
                        nh = 4
                        hw = OUTW // 4
                        for hh in range(nh):
                            nc.vector.tensor_tensor(
                                gt[:, :T, off + hh * hw: off + (hh + 1) * hw],
                                gt[:, :T, off + hh * hw: off + (hh + 1) * hw],
                                gt[:, :T, off + OUTW + hh: off + OUTW + hh + 1
                                   ].to_broadcast([P, T, hw]),
                                AluOpType.mult)

                    # window accumulation
                    for t in range(T):
                        s = s0 + t
                        w = int(slab_win[s])
                        sel = sel_g[:, t]
                        if w not in win_ps:
                            win_ps[w] = pw.tile([P, 512], F32, space="PSUM",
                                                tag="win", name=f"win{w}")
                        st = first_slab[w] == s
                        fin = last_slab[w] == s
                        nc.tensor.matmul(
                            win_ps[w][:, 0:NAW], sel, gt[:, t, 0:NAW],
                            start=st, stop=False, skip_group_check=True)
                        nc.tensor.matmul(
                            win_ps[w][:, 0:NAW], sel,
                            gt[:, t, RWH: RWH + NAW],
                            start=False, stop=fin, skip_group_check=True)
                        if not fin:
                            continue
                        # ---- drain window w
                        pwin = win_ps.pop(w)
                        sk = pk.tile([P, 512], F32, space="PSUM", tag="skps")
                        nc.tensor.matmul(
                            sk[:, 0:64], hTow_sb[:, w * P: (w + 1) * P],
                            skipw_sb[:], start=True, stop=True)
                        rec = ssp.tile([P, 4], F32, tag="rec")
                        nc.vector.reciprocal(rec[:], pwin[:, OUTW: OUTW + 4])
                        yw = y_sb[:, w, :]
                        if L2:
                            m_t = ssp.tile([P, 4, 64], F32, tag="mt")
                            for hh in range(4):
                                nc.vector.tensor_tensor(
                                    m_t[:, hh, :],
                                    pwin[:, hh * 64: (hh + 1) * 64],
                                    rec[:, hh: hh + 1].to_broadcast([P, 64]),
                                    AluOpType.mult)
                            nc.vector.tensor_tensor(yw, m_t[:, 0, :],
                                                    m_t[:, 1, :], AluOpType.add)
                            nc.vector.tensor_tensor(yw, yw, m_t[:, 2, :],
                                                    AluOpType.add)
                            nc.vector.tensor_tensor(yw, yw, m_t[:, 3, :],
                                                    AluOpType.add)
                            nc.vector.tensor_scalar_mul(yw, yw, 0.25)
                            nc.vector.tensor_tensor(yw, yw, sk[:, 0:64],
                                                    AluOpType.add)
                            nc.vector.tensor_tensor(yw, yw, bias_sb[:],
                                                    AluOpType.add)
                        else:
                            for hh in range(4):
                                nc.vector.tensor_tensor(
                                    yw[:, hh * 16: (hh + 1) * 16],
                                    pwin[:, hh * 16: (hh + 1) * 16],
                                    rec[:, hh: hh + 1].to_broadcast([P, 16]),
                                    AluOpType.mult)
                            nc.vector.tensor_tensor(yw, yw, sk[:, 0:64],
                                                    AluOpType.add)
                            nc.vector.tensor_tensor(yw, yw, bias_sb[:],
                                                    AluOpType.add)
                            nc.vector.tensor_scalar_max(yw, yw, 0.0)

            nc.sync.dma_start(
                y_out[:].rearrange("(w p) c -> p w c", p=P), y_sb[:])
    nc.compile()
    return nc


# ------------------------------------------------------------------ driver

_CACHE = {}
_EXEC_NS = []


def _blockdiag(a):
    H, C = a.shape
    m = np.zeros((H * C, H), np.float32)
    for hh in range(H):
        m[hh * C: (hh + 1) * C, hh] = a[hh]
    return m


def kernel(**inp):
    x = np.asarray(inp["x"], np.float32)
    ei = np.asarray(inp["edge_index"], np.int64)
    N, IN = x.shape
    E = ei.shape[1]

    loops = np.arange(N, dtype=np.int64)
    src = np.concatenate([ei[0], loops])
    dst = np.concatenate([ei[1], loops])

    pkey = ("plan", N, E, hash(ei.tobytes()))
    if pkey not in _CACHE:
        _CACHE[pkey] = build_plan(src, dst, N)
    shared, plans = _CACHE[pkey]
    D, nwin = shared["D"], shared["nwin"]
    Dpad = nwin * P
    Npad = _round_up(N, 2 * P)

    def prep01(Wv, a_s, a_d, cb, sW, sb, g, b, m, v):
        Wv, sW = np.asarray(Wv, np.float32), np.asarray(sW, np.float32)
        bns = (np.asarray(g) / np.sqrt(np.asarray(v) + EPS)).astype(np.float32)
        bnt = (np.asarray(b) - np.asarray(m) * bns).astype(np.float32)
        Waug = np.concatenate(
            [Wv * bns[None, :], Wv @ _blockdiag(np.asarray(a_s)),
             Wv @ _blockdiag(np.asarray(a_d))], 1)
        return (Waug, sW * bns[None, :],
                np.asarray(cb) * bns + np.asarray(sb) * bns + bnt)

    def prep2(Wv, a_s, a_d, cb, sW, sb):
        Wv = np.asarray(Wv, np.float32)
        Waug = np.concatenate(
            [Wv, Wv @ _blockdiag(np.asarray(a_s)),
             Wv @ _blockdiag(np.asarray(a_d))], 1)
        return (Waug, np.asarray(sW, np.float32),
                np.asarray(cb) + np.asarray(sb))

    Ls = [
        prep01(inp["conv0_W"], inp["conv0_as"], inp["conv0_ad"], inp["conv0_b"],
               inp["skip0_W"], inp["skip0_b"], inp["bn0_g"], inp["bn0_b"],
               inp["bn0_m"], inp["bn0_v"]),
        prep01(inp["conv1_W"], inp["conv1_as"], inp["conv1_ad"], inp["conv1_b"],
               inp["skip1_W"], inp["skip1_b"], inp["bn1_g"], inp["bn1_b"],
               inp["bn1_m"], inp["bn1_v"]),
        prep2(inp["conv2_W"], inp["conv2_as"], inp["conv2_ad"], inp["conv2_b"],
              inp["skip2_W"], inp["skip2_b"]),
    ]

    iotaF = np.tile(np.arange(P, dtype=np.float32), (P, 1)).astype(ml_dtypes.bfloat16)
    iotaP = np.arange(P, dtype=np.float32).reshape(P, 1).astype(ml_dtypes.bfloat16)

    h = x
    for li in range(3):
        F = IN if li == 0 else 64
        L2 = li == 2
        Waug, skipWf, biasv = Ls[li]
        lkey = ("nc", li, F, N, E)
        if lkey not in _CACHE:
            _CACHE[lkey] = build_layer(
                shared, F, L2, tcap=16 if L2 else 32, gb=4 if L2 else 8)
        nck = _CACHE[lkey]

        hT_full = np.zeros((F, Npad), np.float32)
        hT_full[:, :N] = h.T
        hT_hi = hT_full.astype(ml_dtypes.bfloat16)
        hT_lo = (hT_full - hT_hi.astype(np.float32)).astype(ml_dtypes.bfloat16)
        Waug32 = Waug.astype(np.float32)
        Waug_hi = Waug32.astype(ml_dtypes.bfloat16)
        Waug_lo = (Waug32 - Waug_hi.astype(np.float32)).astype(ml_dtypes.bfloat16)
        base = {
            "hT_hi": hT_hi, "hT_lo": hT_lo,
            "Waug": Waug32, "Waug_hi": Waug_hi, "Waug_lo": Waug_lo,
            "skipW": skipWf.astype(np.float32),
            "biasR": np.tile(biasv.astype(np.float32), (P, 1)),
            "iotaF": iotaF, "iotaP": iotaP,
        }
        in_maps = []
        for c in range(NC):
            pl = plans[c]
            hTow = np.zeros((F, Dpad), np.float32)
            hTow[:, :D] = h[c * D: (c + 1) * D].T
            in_maps.append(dict(base, hTow=hTow, idx=pl["idx"],
                                dposP=pl["dstposP"], dposF=pl["dstposF"],
                                M_E=pl["M_E"], M_O=pl["M_O"]))
        import time as _time
        _t0 = _time.time()
        res = run_bass_kernel_spmd(nck, in_maps, core_ids=list(range(NC)))
        if res.exec_time_ns:
            _EXEC_NS.append(res.exec_time_ns)
        print(f"  layer {li} run wall: {_time.time()-_t0:.1f}s", flush=True)
        hn = np.zeros((N, 64), np.float32)
        for c in range(NC):
            hn[c * D: (c + 1) * D] = res.results[c]["y"][:D]
        h = hn
    return h
